# revision 20
# baseline (speedup 1.0000x reference)
"""Causal self-attention (B=4, T=2048, C=1024, H=16) on 8 NeuronCores.

Sharding: core = (batch b, head-group g): data-parallel over B=4, tensor-
parallel over heads (2 groups x 8 heads).  Each core computes QKV + attention
for its 8 heads and the matching half of the c_proj contraction; the host
sums the two partial c_proj outputs per batch and adds b_proj.

Device layout notes:
  - all matmul operands bf16 (PE runs fp32 at 1/4 rate), PSUM f32
  - x, weights are pre-transposed on the host so every matmul contraction
    sits on the partition dim; no on-device transposes anywhere
  - QKV biases enter as K=1 rank-1 matmuls against a ones row
  - S is computed transposed ([keys, queries]); exp(S/8) on ScalarE with no
    max-subtraction (logits bounded ~+-4 for this problem's scale)
  - causality at tile granularity: k-tiles above the diagonal are skipped,
    diagonal tiles multiplied by precomputed 0/1 masks after exp; diagonal
    tiles further restrict S/exp/PV to their valid column range
  - softmax denominator = ones column appended to each head's V; PV matmul
    emits [y.T | denom] per (head, q-chunk)

Scheduling notes (the performance-critical part):
  - ScalarE exp is the pacer (~0.83ns/col + ~190ns/instr).  S tiles for two
    consecutive k-tiles are written into one [128,1024] 2-bank PSUM tile and
    exp'd with ONE activation for non-diagonal pairs (amortizes the fixed
    overhead); diagonal pairs keep two narrow column-sliced activations.
  - one continuous S->exp->PV pipeline per head across ALL q-chunks (PSUM
    yps double-buffered) so there is no PV-drain bubble at chunk boundaries;
    PV emission lags S by 2 groups (3 for diagonal groups, hiding the GPSIMD
    mask-multiply latency).
  - input DMAs are issued biases -> (wqk[k], xt[k]) pairs -> wv -> masks ->
    wp, and the pair-0 QKV projection is emitted k-innermost so the PE
    consumes tiles as they land; attention starts right after the pair-0
    Q/K projection + first 4 V tiles instead of after the full pre-phase.
  - remaining QKV projection work (pair 1-3 Q/K, V tiles) is drip-fed into
    the attention stream as PE "filler" (keeps the PE dense so the HAM
    activity monitor does not clock-throttle it to 1.2 GHz), with
    deadline-ordered tags force-flushed (ensure) just before a consumer.
  - normalization is per (head, q-chunk): reciprocal of the denominator row
    straight off the copied ysb tile, broadcast to 64 partitions via a
    rank-1 f32r matmul, multiply on DVE; the PE-side broadcast matmul is
    delayed one group so it never waits on the DVE reciprocal.
  - c_proj is emitted per q-chunk of the LAST head as filler work (all
    other pairs' y are normalized long before), so the output projection
    and its DMA overlap the tail of attention instead of serializing.
"""

import os

import numpy as np
import ml_dtypes

B, T, C, H = 4, 2048, 1024, 16
D = 64          # head dim
HL = 8          # heads per core
CL = HL * D     # 512 local channels
TQ = 512        # query chunk (matmul moving dim)
TK = 128        # key tile (psum partition dim)
NQC = T // TQ   # 4 query chunks
NKT = T // TK   # 16 key tiles
VW = HL * (D + 1)  # 520: V with per-head ones column

_prog = None
last_results = None  # BassKernelResults of the most recent run (for test.py)


def _build_program():
    import concourse.mybir as mybir
    import concourse.tile as tile
    from concourse import bacc

    f32 = mybir.dt.float32
    f32r = mybir.dt.float32r
    bf16 = mybir.dt.bfloat16
    EXP = mybir.ActivationFunctionType.Exp

    nc = bacc.Bacc("TRN2", target_bir_lowering=False, debug=False)

    xt_d = nc.dram_tensor("xt", [8, 128, T], bf16, kind="ExternalInput")
    # pair-interleaved: cols 256g..256g+128 = Q pair g, +128..256 = K pair g
    wqk_d = nc.dram_tensor("wqk", [8, 128, 2 * CL], bf16, kind="ExternalInput")
    wv_d = nc.dram_tensor("wv", [8, 128, CL], bf16, kind="ExternalInput")
    # per-pair bias columns, f32: col 2*g = Q pair g, col 2*g+1 = K pair g
    bqk_d = nc.dram_tensor("bqk", [128, 8], f32, kind="ExternalInput")
    wp_d = nc.dram_tensor("wp", [4, 128, C], bf16, kind="ExternalInput")
    mask_d = nc.dram_tensor("mask", [4, 128, TQ], bf16, kind="ExternalInput")
    out_d = nc.dram_tensor("out", [T, C], f32, kind="ExternalOutput")

    with tile.TileContext(nc) as tc:
        with (
            tc.tile_pool(name="persist", bufs=1) as pp,
            tc.tile_pool(name="ptpool", bufs=6) as ptp,
            tc.tile_pool(name="stage", bufs=4) as sp,
            tc.tile_pool(name="small", bufs=3) as smp,
            tc.tile_pool(name="psA", bufs=2, space="PSUM") as psA,   # [128,1024] x2 = 4 banks
            tc.tile_pool(name="psF", bufs=2, space="PSUM") as psF,   # [128,512]  x2 = 2 banks
            tc.tile_pool(name="psY", bufs=2, space="PSUM") as psY,   # [128,512]  x2 = 2 banks
        ):
            # ---- persistent SBUF tensors ----
            xt = [pp.tile([128, T], bf16, name=f"xt{k}") for k in range(8)]
            wqk = [pp.tile([128, 2 * CL], bf16, name=f"wqk{k}") for k in range(8)]
            wv = [pp.tile([128, CL], bf16, name=f"wv{k}") for k in range(8)]
            wp = [pp.tile([128, C], bf16, name=f"wp{k}") for k in range(4)]
            maskt = [pp.tile([128, TQ], bf16, name=f"mask{j}") for j in range(4)]
            bqk_cols = pp.tile([128, 8], f32, name="bqk_cols")
            # one-hot selector matrices: bcast of row 32*s of a [97,512] tile
            # into 64 partitions via a rank-1 matmul (SBUF APs may only start
            # at partition 0/32/64/96, hence the 32-pitch)
            sel = [pp.tile([97, 64], bf16, name=f"sel{i}") for i in range(4)]

            # DMA issue order = deadline order; the tile framework makes each
            # consumer wait only for its own slice, so compute overlaps the
            # load.  The pair-0/j-0 slices the pre-phase consumes are issued
            # as small leading chunks, split across the TWO hwdge issue
            # engines (SP + ACT run their ~600ns-per-dma issue streams in
            # parallel), so the first matmul starts ~5us earlier than a
            # whole-tile wait.
            nc.scalar.dma_start(out=bqk_cols[:], in_=bqk_d[:])
            for k in range(8):
                nc.scalar.dma_start(out=xt[k][:, 0:TQ], in_=xt_d[k][:, 0:TQ])
            for k in range(8):
                nc.sync.dma_start(out=wqk[k][:, 0:256], in_=wqk_d[k][:, 0:256])
            for j in range(4):
                nc.sync.dma_start(out=maskt[j][:], in_=mask_d[j])
            for k in range(8):
                nc.sync.dma_start(out=wv[k][:], in_=wv_d[k])
            for k in range(8):
                nc.sync.dma_start(
                    out=wqk[k][:, 256:], in_=wqk_d[k][:, 256:]
                )
            for k in range(8):
                nc.sync.dma_start(out=xt[k][:, TQ : 2 * TQ],
                                  in_=xt_d[k][:, TQ : 2 * TQ])
            for k in range(4):
                nc.sync.dma_start(out=wp[k][:], in_=wp_d[k])
            for k in range(8):
                nc.sync.dma_start(out=xt[k][:, 2 * TQ :],
                                  in_=xt_d[k][:, 2 * TQ :])
            for i in range(4):
                nc.vector.memset(sel[i][:], 0.0)
                nc.vector.memset(sel[i][32 * i : 32 * i + 1, :], 1.0)

            # QT/KT in [channel, t] layout; channel tile g = head pair g
            qt = [pp.tile([128, T], bf16, name=f"qt{i}") for i in range(4)]
            kt = [pp.tile([128, T], bf16, name=f"kt{i}") for i in range(4)]
            # V in natural [t, channel] layout with a ones column per head
            vsb = [pp.tile([128, VW], bf16, name=f"v{i}") for i in range(NKT)]
            yt = [pp.tile([128, T], bf16, name=f"yt{i}") for i in range(4)]
            # softmax-denominator ones columns: written once here, V copies
            # below use a strided AP that skips them
            for it in range(NKT):
                v3 = vsb[it].rearrange("p (h c) -> p h c", h=HL)
                nc.vector.memset(v3[:, :, D : D + 1], 1.0)

            # ---- pair-0 Q/K j0 projection, k-innermost across the two halves
            # of one wide psA tile, so the PE consumes (wqk[k], xt[k]) as
            # they land from HBM
            def qk_wave(g, j0):
                slot = psA.tile([128, 2 * TQ], f32, name="ps_qk", tag="s_w")
                chs = [(qt, 0, j0), (kt, 1, j0)]
                for k in range(8):
                    for ci, (dst, qk, j) in enumerate(chs):
                        ps = slot[:, ci * TQ : (ci + 1) * TQ]
                        w0 = 256 * g + 128 * qk
                        nc.tensor.matmul(
                            ps,
                            lhsT=wqk[k][:, w0 : w0 + 128],
                            rhs=xt[k][:, j * TQ : (j + 1) * TQ],
                            start=(k == 0),
                            stop=(k == 7),
                        )
                for ci, (dst, qk, j) in enumerate(chs):
                    ps = slot[:, ci * TQ : (ci + 1) * TQ]
                    nc.vector.tensor_scalar_add(
                        dst[g][:, j * TQ : (j + 1) * TQ],
                        ps,
                        bqk_cols[:, 2 * g + qk : 2 * g + qk + 1],
                    )

            def emit_qk_filler(g):
                """One head-pair's Q.T and K.T projection as single-matmul
                closures, tagged per (pair, j) for deadline ensure()."""
                steps = []
                for j in range(NQC):
                    for dst, qk in ((qt, 0), (kt, 1)):
                        ph = {}

                        def step(k, ph=ph, dst=dst, qk=qk, j=j, g=g):
                            if k == 0:
                                ph["ps"] = psF.tile(
                                    [128, TQ], f32, name="ps_f", tag="fill"
                                )
                            if k < 8:
                                w0 = 256 * g + 128 * qk
                                nc.tensor.matmul(
                                    ph["ps"][:],
                                    lhsT=wqk[k][:, w0 : w0 + 128],
                                    rhs=xt[k][:, j * TQ : (j + 1) * TQ],
                                    start=(k == 0),
                                    stop=(k == 7),
                                )
                            else:
                                nc.vector.tensor_scalar_add(
                                    dst[g][:, j * TQ : (j + 1) * TQ],
                                    ph["ps"],
                                    bqk_cols[:, 2 * g + qk : 2 * g + qk + 1],
                                )

                        for k in range(9):
                            steps.append((f"qk{g}j{j}", lambda k=k, step=step: step(k)))
                return steps

            def v_chain_steps(it, h2):
                """V projection for 4 heads of one 128-row t-tile.  The copy
                out uses a 65-pitch strided AP that skips the preset ones
                columns (v bias is folded into b_proj on the host)."""
                ph = {}

                def step(k, ph=ph, it=it, h2=h2):
                    if k == 0:
                        ph["ps"] = psF.tile([128, TQ], f32, name="ps_v", tag="fill")
                    if k < 8:
                        nc.tensor.matmul(
                            ph["ps"][:, : CL // 2],
                            lhsT=xt[k][:, it * 128 : (it + 1) * 128],
                            rhs=wv[k][:, h2 * (CL // 2) : (h2 + 1) * (CL // 2)],
                            start=(k == 0),
                            stop=(k == 7),
                        )
                    else:
                        v3 = vsb[it].rearrange("p (h c) -> p h c", h=HL)
                        ps3 = ph["ps"].rearrange("p (h c) -> p h c", h=HL)
                        nc.vector.tensor_copy(
                            out=v3[:, 4 * h2 : 4 * h2 + 4, 0:D],
                            in_=ps3[:, 0:4, 0:D],
                        )

                return [(f"v{h2}k{it}", lambda k=k, step=step: step(k))
                        for k in range(9)]

            # pre-phase (inline, DMA-paced): pair-0 Q/K chunk j0 + the first
            # 4 V tiles of half 0 — the minimum sweep 0's first PVs consume
            qk_wave(0, 0)
            for it in range(4):
                for _, s in v_chain_steps(it, 0):
                    s()

            # ---- filler queue, deadline-ordered for the qc-major sweeps:
            # sweep s consumes chunk j=s of every pair and V t-tiles 4s..4s+3
            fillers = []
            qk_steps = {g: emit_qk_filler(g) for g in range(4)}
            for g in (1, 2):
                fillers.extend(qk_steps[g][0:18])         # qk{1,2}j0, due h2/h4
            for it in range(4):                           # V half1 0-3, due h4
                fillers.extend(v_chain_steps(it, 1))
            fillers.extend(qk_steps[3][0:18])             # qk3j0, due h6
            for s in range(1, NQC):
                for g in range(4):
                    fillers.extend(qk_steps[g][s * 18 : (s + 1) * 18])
                for it in range(4 * s, 4 * s + 4):
                    for h2 in range(2):
                        fillers.extend(v_chain_steps(it, h2))

            def drain(n):
                for _ in range(n):
                    if fillers:
                        fillers.pop(0)[1]()

            def ensure(tag):
                """Force-emit exactly the closures of `tag` (chains of
                different tags are mutually independent, so skipping others
                is safe and avoids serializing the whole backlog)."""
                keep, run = [], []
                for t, s in fillers:
                    (run if t == tag else keep).append((t, s))
                fillers[:] = keep
                for _, s in run:
                    s()

            # ---- attention: one continuous S->exp->PV stream per head ----
            def c_proj_chain(it, oc):
                """c_proj for one [128,512] output tile: 4-matmul chain +
                copy + DMA, as single-step closures."""
                ph = {}

                def step(s, ph=ph, it=it, oc=oc):
                    if s < 4:
                        if s == 0:
                            ph["ps"] = psF.tile([128, TQ], f32, name="ps_o",
                                                tag="fill")
                        nc.tensor.matmul(
                            ph["ps"][:],
                            lhsT=yt[s][:, it * 128 : (it + 1) * 128],
                            rhs=wp[s][:, oc * TQ : (oc + 1) * TQ],
                            start=(s == 0),
                            stop=(s == 3),
                        )
                    elif s == 4:
                        ph["ot"] = sp.tile([128, TQ], f32, name="ot")
                        nc.vector.tensor_copy(out=ph["ot"][:], in_=ph["ps"][:])
                    else:
                        nc.sync.dma_start(
                            out=out_d[it * 128 : (it + 1) * 128,
                                      oc * TQ : (oc + 1) * TQ],
                            in_=ph["ot"][:],
                        )

                return [("cp", lambda s=s, step=step: step(s)) for s in range(6)]

            # qc-major sweeps: process q-chunk qc for ALL 8 heads, then qc+1.
            # c_proj for a t-range starts one sweep after its columns are
            # final, so the output projection + DMA overlap attention instead
            # of serializing after it.
            pend = []     # (h, qc, g, diag) whose PV is not yet emitted
            yps = {}      # h -> current PSUM y accumulator [65, TQ]
            pts = {}      # (h, qc, g) -> pt tile
            sweep_state = {}  # qc -> (dens[2], slots[2])
            tail_reserve = []  # c_proj chains held for the final norm window

            def norm_steps(grp, qc):
                """Normalization of 4 heads: per-head broadcast+multiply
                closures.  The reciprocals were already computed per head in
                post_pv (straight off the PSUM denominator row), so by the
                time these run the rec tile is complete.  rec is bf16 so the
                rank-1 broadcast matmul runs at full bf16 rate."""
                recs, slots = sweep_state[qc]
                rec_g, group = recs[grp], slots[grp]
                steps = []

                def one(h_, ysb, qc=qc, rec_g=rec_g):
                    g2_, po_ = h_ // 2, 64 * (h_ % 2)
                    bc = psF.tile([64, TQ], f32, name="bc", tag="fill")
                    nc.tensor.matmul(
                        bc[:], lhsT=sel[h_ % 4][:], rhs=rec_g[:],
                        start=True, stop=True,
                    )
                    nc.vector.tensor_mul(
                        yt[g2_][po_ : po_ + 64, qc * TQ : (qc + 1) * TQ],
                        ysb[0:64, :],
                        bc[:],
                    )

                for h_, ysb in group:
                    steps.append(lambda h_=h_, ysb=ysb: one(h_, ysb))
                return steps

            def c_proj_tail(qc):
                """Final sweep's c_proj: direct wide-PSUM chains (the S
                pipeline is done with psA by now), one wide copy + one
                512KB DMA per 128-row t-tile."""
                for p in range(4):
                    it = 4 * qc + p
                    slot = psA.tile([128, 2 * TQ], f32, name="ps_cp",
                                    tag="s_w")
                    for ic in range(4):
                        for oc in range(2):
                            nc.tensor.matmul(
                                slot[:, oc * TQ : (oc + 1) * TQ],
                                lhsT=yt[ic][:, it * 128 : (it + 1) * 128],
                                rhs=wp[ic][:, oc * TQ : (oc + 1) * TQ],
                                start=(ic == 0),
                                stop=(ic == 3),
                            )
                    ot = sp.tile([128, 2 * TQ], f32, name="otw", tag="otw",
                                 bufs=2)
                    for oc in range(2):
                        nc.vector.tensor_copy(
                            out=ot[:, oc * TQ : (oc + 1) * TQ],
                            in_=slot[:, oc * TQ : (oc + 1) * TQ],
                        )
                        nc.sync.dma_start(
                            out=out_d[it * 128 : (it + 1) * 128,
                                      oc * TQ : (oc + 1) * TQ],
                            in_=ot[:, oc * TQ : (oc + 1) * TQ],
                        )

            def post_pv(h, qc):
                """After the last PV of (h, qc): reciprocal of the PSUM
                denominator row straight into this head's row of the group's
                rec tile (no separate den extraction), then copy the head's
                y out of PSUM."""
                recs, slots = sweep_state[qc]
                ysb = smp.tile([D + 1, TQ], f32, name="ysb", tag="ysb",
                               bufs=10)
                yp = yps.pop(h)
                r0 = 32 * (h % 4)
                with nc.allow_low_precision(reason="bf16 softmax recip"):
                    nc.vector.reciprocal(
                        recs[h // 4][r0 : r0 + 1, :], yp[64:65, :]
                    )
                nc.vector.tensor_copy(out=ysb[:], in_=yp[:])
                slots[h // 4].append((h, ysb))
                final = h == HL - 1 and qc == NQC - 1
                if h % 4 == 3 and not final:
                    fillers.extend(("norm", s) for s in norm_steps(h // 4, qc))
                if h == HL - 1:
                    if final:
                        # cover the last head's reciprocal latency with the
                        # reserved c_proj chains so the PE stays hot, then
                        # normalize group 1 and emit the last output tiles
                        for _, s in tail_reserve:
                            s()
                        tail_reserve.clear()
                        drain(8)
                        for s in norm_steps(1, qc):
                            s()
                        c_proj_tail(qc)
                    else:
                        for it in range(4 * qc, 4 * qc + 4):
                            for oc in range(2):
                                steps = c_proj_chain(it, oc)
                                if qc == NQC - 2 and it >= 4 * qc + 2:
                                    tail_reserve.extend(steps)
                                else:
                                    fillers.extend(steps)
                    del sweep_state[qc]

            def pv_group(h, qc, g):
                ktop = (qc + 1) * (TQ // TK)
                pt_w = pts.pop((h, qc, g))
                for hh in range(2):
                    ensure(f"v{h // 4}k{2 * g + hh}")
                for hh in range(2):
                    ktl = 2 * g + hh
                    j = ktl - qc * (TQ // TK)
                    col0 = j * TK if j >= 0 else 0
                    if ktl == 0:
                        yps[h] = psY.tile([D + 1, TQ], f32, name="yps",
                                          tag="y")
                    nc.tensor.matmul(
                        yps[h][:, col0:],
                        lhsT=vsb[ktl][:, h * 65 : (h + 1) * 65],
                        rhs=pt_w[:, hh * TQ + col0 : (hh + 1) * TQ],
                        start=(ktl == 0),
                        stop=(ktl == ktop - 1),
                    )
                if 2 * g + 1 == ktop - 1:
                    post_pv(h, qc)

            def s_group(h, qc, g):
                g2, po = h // 2, 64 * (h % 2)
                diag = 2 * g >= 4 * qc
                ps_s = psA.tile([128, 2 * TQ], f32, name="ps_s", tag="s_w")
                pt_w = ptp.tile([128, 2 * TQ], bf16, name="pt")
                for hh in range(2):
                    ktl = 2 * g + hh
                    j = ktl - qc * (TQ // TK)
                    col0 = j * TK if j >= 0 else 0
                    nc.tensor.matmul(
                        ps_s[:, hh * TQ + col0 : (hh + 1) * TQ],
                        lhsT=kt[g2][po : po + 64, ktl * TK : (ktl + 1) * TK],
                        rhs=qt[g2][po : po + 64,
                                   qc * TQ + col0 : (qc + 1) * TQ],
                        start=True,
                        stop=True,
                    )
                if not diag:
                    # one wide exp over both k-tiles (2 PSUM banks)
                    nc.scalar.activation(pt_w[:, :], ps_s[:, :], EXP,
                                         scale=0.125)
                else:
                    for hh in range(2):
                        ktl = 2 * g + hh
                        j = ktl - qc * (TQ // TK)
                        col0 = j * TK
                        nc.scalar.activation(
                            pt_w[:, hh * TQ + col0 : (hh + 1) * TQ],
                            ps_s[:, hh * TQ + col0 : (hh + 1) * TQ],
                            EXP,
                            scale=0.125,
                        )
                        nc.gpsimd.tensor_mul(
                            pt_w[:, hh * TQ + col0 : (hh + 1) * TQ],
                            pt_w[:, hh * TQ + col0 : (hh + 1) * TQ],
                            maskt[j][:, col0:],
                        )
                pts[(h, qc, g)] = pt_w
                return diag

            def tick():
                """After each S group emission: drip fillers, emit lagged PV
                groups (deeper lag for diagonal groups hides mask latency)."""
                drain(4 if len(fillers) > 160 else 3 if len(fillers) > 40
                      else 2)
                if pend and len(pend) >= (3 if pend[0][3] else 2):
                    h0, qc0, g0, _ = pend.pop(0)
                    pv_group(h0, qc0, g0)

            for qc in range(NQC):
                # flush previous sweeps' normalization before re-using the
                # rec/ysb slots (keeps the in-order DVE queue acyclic)
                ensure("norm")
                recs = [smp.tile([97, TQ], bf16, name=f"rec{i}", tag="recg",
                                 bufs=4) for i in range(2)]
                for rg in recs:
                    nc.vector.memset(rg[:], 1.0)  # rows between heads unused
                sweep_state[qc] = (recs, [[], []])
                for h in range(HL):
                    g2 = h // 2
                    ensure(f"qk{g2}j{qc}")
                    # spread V-tile prefetch over the heads whose PVs lag
                    if h in (0, 1, 4, 5):
                        half, o = h // 4, 2 * (h % 4)
                        ensure(f"v{half}k{4 * qc + o}")
                        ensure(f"v{half}k{4 * qc + o + 1}")
                    # prefetch next sweep's projection chunks while this
                    # sweep's exp stream can still hide the PE work
                    if qc < NQC - 1 and h >= 4:
                        ensure(f"qk{h - 4}j{qc + 1}")
                    for g in range((qc + 1) * (TQ // TK) // 2):
                        diag = s_group(h, qc, g)
                        pend.append((h, qc, g, diag))
                        tick()
            while pend:
                h0, qc0, g0, _ = pend.pop(0)
                pv_group(h0, qc0, g0)
                drain(1)

            drain(len(fillers))

    nc.finalize()
    return nc


def _bf16(a):
    return np.ascontiguousarray(a, dtype=np.float32).astype(ml_dtypes.bfloat16)


def _core_inputs(x, w_attn, b_attn, w_proj, masks, core):
    b, g = divmod(core, 2)
    gs = slice(g * CL, (g + 1) * CL)
    wq, wk, wv_ = (w_attn[i * C : (i + 1) * C][gs] for i in range(3))
    bq, bk = (b_attn[i * C : (i + 1) * C][gs] for i in range(2))

    # pair-interleaved QKV weight: col block 2p = Q pair p, 2p+1 = K pair p
    wqkT = np.empty((C, 2 * CL), np.float32)
    for p in range(4):
        wqkT[:, 256 * p : 256 * p + 128] = wq.T[:, 128 * p : 128 * (p + 1)]
        wqkT[:, 256 * p + 128 : 256 * (p + 1)] = wk.T[:, 128 * p : 128 * (p + 1)]
    # f32 bias columns, col 2p = Q pair p, col 2p+1 = K pair p
    bqk_cols = np.empty((128, 8), np.float32)
    for p in range(4):
        bqk_cols[:, 2 * p] = bq[128 * p : 128 * (p + 1)]
        bqk_cols[:, 2 * p + 1] = bk[128 * p : 128 * (p + 1)]

    return {
        "xt": _bf16(x[b].T).reshape(8, 128, T),
        "wqk": _bf16(wqkT).reshape(8, 128, 2 * CL),
        "wv": _bf16(wv_.T).reshape(8, 128, CL),
        "bqk": bqk_cols,
        "wp": _bf16(w_proj[:, gs].T).reshape(4, 128, C),
        "mask": masks,
    }


def _make_masks():
    qq = np.arange(TQ)[None, :]
    kk = np.arange(TK)[:, None]
    m = np.stack([(qq >= kk + j * TK) for j in range(4)]).astype(np.float32)
    return m.astype(ml_dtypes.bfloat16)


def kernel(x, w_attn, b_attn, w_proj, b_proj):
    global _prog, last_results
    from concourse.bass_utils import run_bass_kernel_spmd

    if _prog is None:
        _prog = _build_program()

    x = np.asarray(x, np.float32)
    w_attn = np.asarray(w_attn, np.float32)
    b_attn = np.asarray(b_attn, np.float32)
    w_proj = np.asarray(w_proj, np.float32)
    b_proj = np.asarray(b_proj, np.float32)

    masks = _make_masks()
    in_maps = [
        _core_inputs(x, w_attn, b_attn, w_proj, masks, core) for core in range(8)
    ]
    kwargs = {}
    tmpdir = os.environ.get("BASS_TMPDIR")
    if tmpdir:
        os.makedirs(tmpdir, exist_ok=True)
        kwargs["tmpdir"] = tmpdir
    res = run_bass_kernel_spmd(_prog, in_maps, list(range(8)), **kwargs)
    last_results = res

    # v-bias passes through attention as a constant (softmax rows sum to 1),
    # so its c_proj image is folded into the host-side bias add
    b_eff = b_proj + b_attn[2 * C :] @ w_proj.T
    out = np.empty((B, T, C), np.float32)
    for b in range(B):
        out[b] = res.results[2 * b]["out"] + res.results[2 * b + 1]["out"] + b_eff
    return out



# revision 36
# speedup vs baseline: 1.3335x; 1.3335x over previous
"""Causal self-attention (B=4, T=2048, C=1024, H=16) on 8 NeuronCores.

Sharding: core = (batch b, head-group g): data-parallel over B=4, tensor-
parallel over heads (2 groups x 8 heads).  Each core computes QKV + attention
for its 8 heads and the matching half of the c_proj contraction; the host
sums the two partial c_proj outputs per batch and adds b_proj.

Device layout notes:
  - all matmul operands bf16 (PE runs fp32 at 1/4 rate), PSUM f32
  - x, weights are pre-transposed on the host so every matmul contraction
    sits on the partition dim; no on-device transposes anywhere
  - QKV biases enter as K=1 rank-1 matmuls against a ones row
  - S is computed transposed ([keys, queries]); exp(S/8) on ScalarE with no
    max-subtraction (logits bounded ~+-4 for this problem's scale)
  - causality at tile granularity: k-tiles above the diagonal are skipped,
    diagonal tiles multiplied by precomputed 0/1 masks after exp; diagonal
    tiles further restrict S/exp/PV to their valid column range
  - softmax denominator = ones column appended to each head's V; PV matmul
    emits [y.T | denom] per (head, q-chunk)

Scheduling notes (the performance-critical part):
  - ScalarE exp is the pacer (~0.83ns/col + ~190ns/instr).  S tiles for two
    consecutive k-tiles are written into one [128,1024] 2-bank PSUM tile and
    exp'd with ONE activation for non-diagonal pairs (amortizes the fixed
    overhead); diagonal pairs keep two narrow column-sliced activations.
  - one continuous S->exp->PV pipeline per head across ALL q-chunks (PSUM
    yps double-buffered) so there is no PV-drain bubble at chunk boundaries;
    PV emission lags S by 2 groups (3 for diagonal groups, hiding the GPSIMD
    mask-multiply latency).
  - input DMAs are issued biases -> (wqk[k], xt[k]) pairs -> wv -> masks ->
    wp, and the pair-0 QKV projection is emitted k-innermost so the PE
    consumes tiles as they land; attention starts right after the pair-0
    Q/K projection + first 4 V tiles instead of after the full pre-phase.
  - remaining QKV projection work (pair 1-3 Q/K, V tiles) is drip-fed into
    the attention stream as PE "filler" (keeps the PE dense so the HAM
    activity monitor does not clock-throttle it to 1.2 GHz), with
    deadline-ordered tags force-flushed (ensure) just before a consumer.
  - normalization is per (head, q-chunk): reciprocal of the denominator row
    straight off the copied ysb tile, broadcast to 64 partitions via a
    rank-1 f32r matmul, multiply on DVE; the PE-side broadcast matmul is
    delayed one group so it never waits on the DVE reciprocal.
  - c_proj is emitted per q-chunk of the LAST head as filler work (all
    other pairs' y are normalized long before), so the output projection
    and its DMA overlap the tail of attention instead of serializing.
"""

import os

import numpy as np
import ml_dtypes

B, T, C, H = 4, 2048, 1024, 16
D = 64          # head dim
HL = 8          # heads per core
CL = HL * D     # 512 local channels
TQ = 512        # query chunk (matmul moving dim)
TK = 128        # key tile (psum partition dim)
NQC = T // TQ   # 4 query chunks
NKT = T // TK   # 16 key tiles
VW = HL * (D + 1)  # 520: V with per-head ones column

_prog = None
last_results = None  # BassKernelResults of the most recent run (for test.py)


def _build_program():
    import concourse.mybir as mybir
    import concourse.tile as tile
    from concourse import bacc

    f32 = mybir.dt.float32
    f32r = mybir.dt.float32r
    bf16 = mybir.dt.bfloat16
    EXP = mybir.ActivationFunctionType.Exp

    nc = bacc.Bacc("TRN2", target_bir_lowering=False, debug=False)

    xt_d = nc.dram_tensor("xt", [8, 128, T], bf16, kind="ExternalInput")
    # pair-interleaved: cols 256g..256g+128 = Q pair g, +128..256 = K pair g
    wqk_d = nc.dram_tensor("wqk", [8, 128, 2 * CL], bf16, kind="ExternalInput")
    wv_d = nc.dram_tensor("wv", [8, 128, CL], bf16, kind="ExternalInput")
    # per-pair bias columns, f32: col 2*g = Q pair g, col 2*g+1 = K pair g
    bqk_d = nc.dram_tensor("bqk", [128, 8], f32, kind="ExternalInput")
    wp_d = nc.dram_tensor("wp", [4, 128, C], bf16, kind="ExternalInput")
    mask_d = nc.dram_tensor("mask", [4, 128, TQ], bf16, kind="ExternalInput")
    out_d = nc.dram_tensor("out", [T, C], f32, kind="ExternalOutput")

    with tile.TileContext(nc) as tc:
        with (
            tc.tile_pool(name="persist", bufs=1) as pp,
            tc.tile_pool(name="ptpool", bufs=6) as ptp,
            tc.tile_pool(name="stage", bufs=4) as sp,
            tc.tile_pool(name="small", bufs=3) as smp,
            tc.tile_pool(name="psA", bufs=2, space="PSUM") as psA,   # [128,1024] x2 = 4 banks
            tc.tile_pool(name="psF", bufs=2, space="PSUM") as psF,   # [128,512]  x2 = 2 banks
            tc.tile_pool(name="psY", bufs=2, space="PSUM") as psY,   # [128,512]  x2 = 2 banks
        ):
            # ---- persistent SBUF tensors ----
            xt = [pp.tile([128, T], bf16, name=f"xt{k}") for k in range(8)]
            wqk = [pp.tile([128, 2 * CL], bf16, name=f"wqk{k}") for k in range(8)]
            wv = [pp.tile([128, CL], bf16, name=f"wv{k}") for k in range(8)]
            wp = [pp.tile([128, C], bf16, name=f"wp{k}") for k in range(4)]
            maskt = [pp.tile([128, TQ], bf16, name=f"mask{j}") for j in range(4)]
            bqk_cols = pp.tile([128, 8], f32, name="bqk_cols")
            # one-hot selector matrices: bcast of row 32*s of a [97,512] tile
            # into 64 partitions via a rank-1 matmul (SBUF APs may only start
            # at partition 0/32/64/96, hence the 32-pitch)
            sel = [pp.tile([97, 64], bf16, name=f"sel{i}") for i in range(4)]

            # DMA issue order = deadline order; the tile framework makes each
            # consumer wait only for its own slice, so compute overlaps the
            # load.  The pair-0/j-0 slices the pre-phase consumes are issued
            # as small leading chunks, split across the TWO hwdge issue
            # engines (SP + ACT run their ~600ns-per-dma issue streams in
            # parallel), so the first matmul starts ~5us earlier than a
            # whole-tile wait.
            nc.scalar.dma_start(out=bqk_cols[:], in_=bqk_d[:])
            for k in range(8):
                nc.scalar.dma_start(out=xt[k][:, 0:TQ], in_=xt_d[k][:, 0:TQ])
            for k in range(4):
                nc.scalar.dma_start(out=wv[k][:], in_=wv_d[k])
            for k in range(8):
                nc.sync.dma_start(out=wqk[k][:, 0:256], in_=wqk_d[k][:, 0:256])
            for k in range(4, 8):
                nc.sync.dma_start(out=wv[k][:], in_=wv_d[k])
            for j in range(4):
                nc.sync.dma_start(out=maskt[j][:], in_=mask_d[j])
            for k in range(8):
                nc.sync.dma_start(
                    out=wqk[k][:, 256:], in_=wqk_d[k][:, 256:]
                )
            for k in range(8):
                nc.sync.dma_start(out=xt[k][:, TQ : 2 * TQ],
                                  in_=xt_d[k][:, TQ : 2 * TQ])
            for k in range(4):
                nc.sync.dma_start(out=wp[k][:], in_=wp_d[k])
            for k in range(8):
                nc.sync.dma_start(out=xt[k][:, 2 * TQ :],
                                  in_=xt_d[k][:, 2 * TQ :])
            for i in range(4):
                nc.vector.memset(sel[i][:], 0.0)
                nc.vector.memset(sel[i][32 * i : 32 * i + 1, :], 1.0)

            # QT/KT in [channel, t] layout; channel tile g = head pair g
            qt = [pp.tile([128, T], bf16, name=f"qt{i}") for i in range(4)]
            kt = [pp.tile([128, T], bf16, name=f"kt{i}") for i in range(4)]
            # V in natural [t, channel] layout with a ones column per head
            vsb = [pp.tile([128, VW], bf16, name=f"v{i}") for i in range(NKT)]
            yt = [pp.tile([128, T], bf16, name=f"yt{i}") for i in range(4)]
            # softmax-denominator ones columns: written once here, V copies
            # below use a strided AP that skips them
            for it in range(NKT):
                v3 = vsb[it].rearrange("p (h c) -> p h c", h=HL)
                nc.vector.memset(v3[:, :, D : D + 1], 1.0)

            # ---- pair-0 Q/K j0 projection, k-innermost across the two halves
            # of one wide psA tile, so the PE consumes (wqk[k], xt[k]) as
            # they land from HBM
            def qk_wave(g, j0):
                slot = psA.tile([128, 2 * TQ], f32, name="ps_qk", tag="s_w")
                chs = [(qt, 0, j0), (kt, 1, j0)]
                for k in range(8):
                    for ci, (dst, qk, j) in enumerate(chs):
                        ps = slot[:, ci * TQ : (ci + 1) * TQ]
                        w0 = 256 * g + 128 * qk
                        nc.tensor.matmul(
                            ps,
                            lhsT=wqk[k][:, w0 : w0 + 128],
                            rhs=xt[k][:, j * TQ : (j + 1) * TQ],
                            start=(k == 0),
                            stop=(k == 7),
                        )
                for ci, (dst, qk, j) in enumerate(chs):
                    ps = slot[:, ci * TQ : (ci + 1) * TQ]
                    nc.vector.tensor_scalar_add(
                        dst[g][:, j * TQ : (j + 1) * TQ],
                        ps,
                        bqk_cols[:, 2 * g + qk : 2 * g + qk + 1],
                    )

            def emit_qk_filler(g):
                """One head-pair's Q.T and K.T projection as single-matmul
                closures, tagged per (pair, j) for deadline ensure()."""
                steps = []
                for j in range(NQC):
                    for dst, qk in ((qt, 0), (kt, 1)):
                        ph = {}

                        def step(k, ph=ph, dst=dst, qk=qk, j=j, g=g):
                            if k == 0:
                                ph["ps"] = psF.tile(
                                    [128, TQ], f32, name="ps_f", tag="fill"
                                )
                            if k < 8:
                                w0 = 256 * g + 128 * qk
                                nc.tensor.matmul(
                                    ph["ps"][:],
                                    lhsT=wqk[k][:, w0 : w0 + 128],
                                    rhs=xt[k][:, j * TQ : (j + 1) * TQ],
                                    start=(k == 0),
                                    stop=(k == 7),
                                )
                            else:
                                nc.vector.tensor_scalar_add(
                                    dst[g][:, j * TQ : (j + 1) * TQ],
                                    ph["ps"],
                                    bqk_cols[:, 2 * g + qk : 2 * g + qk + 1],
                                )

                        for k in range(9):
                            steps.append((f"qk{g}j{j}", lambda k=k, step=step: step(k)))
                return steps

            def v_chain_steps(it, h2):
                """V projection for 4 heads of one 128-row t-tile.  The copy
                out uses a 65-pitch strided AP that skips the preset ones
                columns (v bias is folded into b_proj on the host)."""
                ph = {}

                def step(k, ph=ph, it=it, h2=h2):
                    if k == 0:
                        ph["ps"] = psF.tile([128, TQ], f32, name="ps_v", tag="fill")
                    if k < 8:
                        nc.tensor.matmul(
                            ph["ps"][:, : CL // 2],
                            lhsT=xt[k][:, it * 128 : (it + 1) * 128],
                            rhs=wv[k][:, h2 * (CL // 2) : (h2 + 1) * (CL // 2)],
                            start=(k == 0),
                            stop=(k == 7),
                        )
                    else:
                        v3 = vsb[it].rearrange("p (h c) -> p h c", h=HL)
                        ps3 = ph["ps"].rearrange("p (h c) -> p h c", h=HL)
                        nc.vector.tensor_copy(
                            out=v3[:, 4 * h2 : 4 * h2 + 4, 0:D],
                            in_=ps3[:, 0:4, 0:D],
                        )

                return [(f"v{h2}k{it}", lambda k=k, step=step: step(k))
                        for k in range(9)]

            # pre-phase (inline, DMA-paced): pair-0 Q/K chunk j0 + the first
            # 4 V tiles of half 0 — the minimum sweep 0's first PVs consume
            qk_wave(0, 0)
            for it in range(4):
                for _, s in v_chain_steps(it, 0):
                    s()

            # ---- filler queue, deadline-ordered for the qc-major sweeps:
            # sweep s consumes chunk j=s of every pair and V t-tiles 4s..4s+3
            fillers = []
            qk_steps = {g: emit_qk_filler(g) for g in range(4)}
            for g in (1, 2):
                fillers.extend(qk_steps[g][0:18])         # qk{1,2}j0, due h2/h4
            for it in range(4):                           # V half1 0-3, due h4
                fillers.extend(v_chain_steps(it, 1))
            fillers.extend(qk_steps[3][0:18])             # qk3j0, due h6
            for s in range(1, NQC):
                for g in range(4):
                    fillers.extend(qk_steps[g][s * 18 : (s + 1) * 18])
                for it in range(4 * s, 4 * s + 4):
                    for h2 in range(2):
                        fillers.extend(v_chain_steps(it, h2))

            def drain(n):
                for _ in range(n):
                    if fillers:
                        fillers.pop(0)[1]()

            def ensure(tag):
                """Force-emit exactly the closures of `tag` (chains of
                different tags are mutually independent, so skipping others
                is safe and avoids serializing the whole backlog)."""
                keep, run = [], []
                for t, s in fillers:
                    (run if t == tag else keep).append((t, s))
                fillers[:] = keep
                for _, s in run:
                    s()

            # ---- attention: one continuous S->exp->PV stream per head ----
            def c_proj_chain(it, oc):
                """c_proj for one [128,512] output tile: 4-matmul chain +
                copy + DMA, as single-step closures."""
                ph = {}

                def step(s, ph=ph, it=it, oc=oc):
                    if s < 4:
                        if s == 0:
                            ph["ps"] = psF.tile([128, TQ], f32, name="ps_o",
                                                tag="fill")
                        nc.tensor.matmul(
                            ph["ps"][:],
                            lhsT=yt[s][:, it * 128 : (it + 1) * 128],
                            rhs=wp[s][:, oc * TQ : (oc + 1) * TQ],
                            start=(s == 0),
                            stop=(s == 3),
                        )
                    elif s == 4:
                        ph["ot"] = sp.tile([128, TQ], f32, name="ot")
                        nc.vector.tensor_copy(out=ph["ot"][:], in_=ph["ps"][:])
                    else:
                        nc.sync.dma_start(
                            out=out_d[it * 128 : (it + 1) * 128,
                                      oc * TQ : (oc + 1) * TQ],
                            in_=ph["ot"][:],
                        )

                return [("cp", lambda s=s, step=step: step(s)) for s in range(6)]

            # qc-major sweeps: process q-chunk qc for ALL 8 heads, then qc+1.
            # c_proj for a t-range starts one sweep after its columns are
            # final, so the output projection + DMA overlap attention instead
            # of serializing after it.
            pend = []     # (h, qc, g, diag) whose PV is not yet emitted
            yps = {}      # h -> current PSUM y accumulator [65, TQ]
            pts = {}      # (h, qc, g) -> pt tile
            sweep_state = {}  # qc -> (dens[2], slots[2])
            tail_reserve = []  # c_proj chains held for the final norm window

            def norm_steps(grp, qc):
                """Normalization of 4 heads: one full-tile approx reciprocal
                of the batched den tile (rows filled per head in post_pv,
                straight off the PSUM denominator row), one f32->bf16 cast,
                then per-head broadcast+multiply closures.  bf16 rec keeps
                the rank-1 broadcast matmul at full bf16 rate."""
                dens, slots = sweep_state[qc]
                den_g, group = dens[grp], slots[grp]
                rec_g = smp.tile([97, TQ], f32, name="rec_g", tag="recg",
                                 bufs=3)
                rec_b = smp.tile([97, TQ], bf16, name="rec_b", tag="recb",
                                 bufs=3)
                steps = [
                    lambda: nc.vector.reciprocal_approx_fast(
                        out=rec_g[:], in_=den_g[:]),
                    lambda: nc.vector.tensor_copy(out=rec_b[:], in_=rec_g[:]),
                ]

                def one(h_, ysb, qc=qc, rec_b=rec_b):
                    g2_, po_ = h_ // 2, 64 * (h_ % 2)
                    bc = psF.tile([64, TQ], f32, name="bc", tag="fill")
                    nc.tensor.matmul(
                        bc[:], lhsT=sel[h_ % 4][:], rhs=rec_b[:],
                        start=True, stop=True,
                    )
                    nc.vector.tensor_mul(
                        yt[g2_][po_ : po_ + 64, qc * TQ : (qc + 1) * TQ],
                        ysb[0:64, :],
                        bc[:],
                    )

                for h_, ysb in group:
                    steps.append(lambda h_=h_, ysb=ysb: one(h_, ysb))
                return steps

            def c_proj_tail(qc, norm_tail):
                """Final sweep's c_proj on wide PSUM (the S pipeline is done
                with psA by now).  Pairs 0/1 of the contraction only need
                group-0 heads (normalized long ago), so two tiles' worth of
                those matmuls are emitted first and the final group's
                broadcast+multiply steps (`norm_tail`) are woven between
                them to hide the normalization round-trip latency."""
                slots = {}

                def front_half(p):
                    it = 4 * qc + p
                    slots[p] = psA.tile([128, 2 * TQ], f32, name="ps_cp",
                                        tag="s_w")
                    for ic in range(2):
                        for oc in range(2):
                            nc.tensor.matmul(
                                slots[p][:, oc * TQ : (oc + 1) * TQ],
                                lhsT=yt[ic][:, it * 128 : (it + 1) * 128],
                                rhs=wp[ic][:, oc * TQ : (oc + 1) * TQ],
                                start=(ic == 0),
                                stop=False,
                            )

                def back_half(p):
                    it = 4 * qc + p
                    for ic in range(2, 4):
                        for oc in range(2):
                            nc.tensor.matmul(
                                slots[p][:, oc * TQ : (oc + 1) * TQ],
                                lhsT=yt[ic][:, it * 128 : (it + 1) * 128],
                                rhs=wp[ic][:, oc * TQ : (oc + 1) * TQ],
                                start=False,
                                stop=(ic == 3),
                            )
                    ot = sp.tile([128, 2 * TQ], f32, name="otw", tag="otw",
                                 bufs=2)
                    for oc in range(2):
                        nc.vector.tensor_copy(
                            out=ot[:, oc * TQ : (oc + 1) * TQ],
                            in_=slots[p][:, oc * TQ : (oc + 1) * TQ],
                        )
                        nc.sync.dma_start(
                            out=out_d[it * 128 : (it + 1) * 128,
                                      oc * TQ : (oc + 1) * TQ],
                            in_=ot[:, oc * TQ : (oc + 1) * TQ],
                        )

                norm_tail = list(norm_tail)
                front_half(0)
                if norm_tail:
                    norm_tail.pop(0)()          # bcast+mul head 4
                front_half(1)
                while norm_tail:
                    norm_tail.pop(0)()          # bcast+mul heads 5-7
                for p in range(2):
                    back_half(p)
                for p in range(2, 4):
                    front_half(p)
                    back_half(p)

            def post_pv(h, qc):
                """After the last PV of (h, qc): copy the PSUM denominator
                row into this head's row of the group's batched den tile,
                then copy the head's y out of PSUM."""
                dens, slots = sweep_state[qc]
                ysb = smp.tile([D + 1, TQ], f32, name="ysb", tag="ysb",
                               bufs=10)
                yp = yps.pop(h)
                r0 = 32 * (h % 4)
                nc.vector.tensor_copy(
                    out=dens[h // 4][r0 : r0 + 1, :], in_=yp[64:65, :]
                )
                nc.vector.tensor_copy(out=ysb[:], in_=yp[:])
                slots[h // 4].append((h, ysb))
                final = h == HL - 1 and qc == NQC - 1
                if h % 4 == 3 and not final:
                    fillers.extend(("norm", s) for s in norm_steps(h // 4, qc))
                if h == HL - 1:
                    if final:
                        # cover the last head's reciprocal latency with the
                        # reserved c_proj chains so the PE stays hot, then
                        # weave group 1's normalization into the first tail
                        # c_proj chains (whose pair-0/1 contractions only
                        # need the long-finished group-0 heads)
                        steps = norm_steps(1, qc)
                        steps[0](), steps[1]()      # recip + cast
                        for _, s in tail_reserve:
                            s()
                        tail_reserve.clear()
                        drain(8)
                        c_proj_tail(qc, steps[2:])
                    else:
                        for it in range(4 * qc, 4 * qc + 4):
                            for oc in range(2):
                                steps = c_proj_chain(it, oc)
                                if qc == NQC - 2 and it >= 4 * qc + 2:
                                    tail_reserve.extend(steps)
                                else:
                                    fillers.extend(steps)
                    del sweep_state[qc]

            def pv_group(h, qc, g):
                ktop = (qc + 1) * (TQ // TK)
                pt_w = pts.pop((h, qc, g))
                for hh in range(2):
                    ensure(f"v{h // 4}k{2 * g + hh}")
                for hh in range(2):
                    ktl = 2 * g + hh
                    j = ktl - qc * (TQ // TK)
                    col0 = j * TK if j >= 0 else 0
                    if ktl == 0:
                        yps[h] = psY.tile([D + 1, TQ], f32, name="yps",
                                          tag="y")
                    nc.tensor.matmul(
                        yps[h][:, col0:],
                        lhsT=vsb[ktl][:, h * 65 : (h + 1) * 65],
                        rhs=pt_w[:, hh * TQ + col0 : (hh + 1) * TQ],
                        start=(ktl == 0),
                        stop=(ktl == ktop - 1),
                    )
                if 2 * g + 1 == ktop - 1:
                    post_pv(h, qc)

            def s_group(h, qc, g):
                g2, po = h // 2, 64 * (h % 2)
                diag = 2 * g >= 4 * qc
                ps_s = psA.tile([128, 2 * TQ], f32, name="ps_s", tag="s_w")
                pt_w = ptp.tile([128, 2 * TQ], bf16, name="pt")
                for hh in range(2):
                    ktl = 2 * g + hh
                    j = ktl - qc * (TQ // TK)
                    col0 = j * TK if j >= 0 else 0
                    nc.tensor.matmul(
                        ps_s[:, hh * TQ + col0 : (hh + 1) * TQ],
                        lhsT=kt[g2][po : po + 64, ktl * TK : (ktl + 1) * TK],
                        rhs=qt[g2][po : po + 64,
                                   qc * TQ + col0 : (qc + 1) * TQ],
                        start=True,
                        stop=True,
                    )
                if not diag:
                    # one wide exp over both k-tiles (2 PSUM banks)
                    nc.scalar.activation(pt_w[:, :], ps_s[:, :], EXP,
                                         scale=0.125)
                else:
                    for hh in range(2):
                        ktl = 2 * g + hh
                        j = ktl - qc * (TQ // TK)
                        col0 = j * TK
                        nc.scalar.activation(
                            pt_w[:, hh * TQ + col0 : (hh + 1) * TQ],
                            ps_s[:, hh * TQ + col0 : (hh + 1) * TQ],
                            EXP,
                            scale=0.125,
                        )
                        nc.gpsimd.tensor_mul(
                            pt_w[:, hh * TQ + col0 : (hh + 1) * TQ],
                            pt_w[:, hh * TQ + col0 : (hh + 1) * TQ],
                            maskt[j][:, col0:],
                        )
                pts[(h, qc, g)] = pt_w
                return diag

            def tick():
                """After each S group emission: drip fillers, emit lagged PV
                groups (deeper lag for diagonal groups hides mask latency)."""
                drain(4 if len(fillers) > 160 else 3 if len(fillers) > 40
                      else 2)
                if pend and len(pend) >= (3 if pend[0][3] else 2):
                    h0, qc0, g0, _ = pend.pop(0)
                    pv_group(h0, qc0, g0)

            for qc in range(NQC):
                # flush previous sweeps' normalization before re-using the
                # rec/ysb slots (keeps the in-order DVE queue acyclic)
                ensure("norm")
                dens = [smp.tile([97, TQ], f32, name=f"den{i}", tag="deng",
                                 bufs=4) for i in range(2)]
                for dg in dens:
                    nc.vector.memset(dg[:], 1.0)  # rows between heads unused
                sweep_state[qc] = (dens, [[], []])
                for h in range(HL):
                    g2 = h // 2
                    ensure(f"qk{g2}j{qc}")
                    # spread V-tile prefetch over the heads whose PVs lag
                    if h in (0, 1, 4, 5):
                        half, o = h // 4, 2 * (h % 4)
                        ensure(f"v{half}k{4 * qc + o}")
                        ensure(f"v{half}k{4 * qc + o + 1}")
                    # prefetch next sweep's projection chunks while this
                    # sweep's exp stream can still hide the PE work
                    if qc < NQC - 1 and h >= 4:
                        ensure(f"qk{h - 4}j{qc + 1}")
                    for g in range((qc + 1) * (TQ // TK) // 2):
                        diag = s_group(h, qc, g)
                        pend.append((h, qc, g, diag))
                        tick()
            while pend:
                h0, qc0, g0, _ = pend.pop(0)
                pv_group(h0, qc0, g0)
                drain(1)

            drain(len(fillers))

    nc.finalize()
    return nc


def _bf16(a):
    return np.ascontiguousarray(a, dtype=np.float32).astype(ml_dtypes.bfloat16)


def _core_inputs(x, w_attn, b_attn, w_proj, masks, core):
    b, g = divmod(core, 2)
    gs = slice(g * CL, (g + 1) * CL)
    wq, wk, wv_ = (w_attn[i * C : (i + 1) * C][gs] for i in range(3))
    bq, bk = (b_attn[i * C : (i + 1) * C][gs] for i in range(2))

    # pair-interleaved QKV weight: col block 2p = Q pair p, 2p+1 = K pair p
    wqkT = np.empty((C, 2 * CL), np.float32)
    for p in range(4):
        wqkT[:, 256 * p : 256 * p + 128] = wq.T[:, 128 * p : 128 * (p + 1)]
        wqkT[:, 256 * p + 128 : 256 * (p + 1)] = wk.T[:, 128 * p : 128 * (p + 1)]
    # f32 bias columns, col 2p = Q pair p, col 2p+1 = K pair p
    bqk_cols = np.empty((128, 8), np.float32)
    for p in range(4):
        bqk_cols[:, 2 * p] = bq[128 * p : 128 * (p + 1)]
        bqk_cols[:, 2 * p + 1] = bk[128 * p : 128 * (p + 1)]

    return {
        "xt": _bf16(x[b].T).reshape(8, 128, T),
        "wqk": _bf16(wqkT).reshape(8, 128, 2 * CL),
        "wv": _bf16(wv_.T).reshape(8, 128, CL),
        "bqk": bqk_cols,
        "wp": _bf16(w_proj[:, gs].T).reshape(4, 128, C),
        "mask": masks,
    }


def _make_masks():
    qq = np.arange(TQ)[None, :]
    kk = np.arange(TK)[:, None]
    m = np.stack([(qq >= kk + j * TK) for j in range(4)]).astype(np.float32)
    return m.astype(ml_dtypes.bfloat16)


def kernel(x, w_attn, b_attn, w_proj, b_proj):
    global _prog, last_results
    from concourse.bass_utils import run_bass_kernel_spmd

    if _prog is None:
        _prog = _build_program()

    x = np.asarray(x, np.float32)
    w_attn = np.asarray(w_attn, np.float32)
    b_attn = np.asarray(b_attn, np.float32)
    w_proj = np.asarray(w_proj, np.float32)
    b_proj = np.asarray(b_proj, np.float32)

    masks = _make_masks()
    in_maps = [
        _core_inputs(x, w_attn, b_attn, w_proj, masks, core) for core in range(8)
    ]
    kwargs = {}
    tmpdir = os.environ.get("BASS_TMPDIR")
    if tmpdir:
        os.makedirs(tmpdir, exist_ok=True)
        kwargs["tmpdir"] = tmpdir
    res = run_bass_kernel_spmd(_prog, in_maps, list(range(8)), **kwargs)
    last_results = res

    # v-bias passes through attention as a constant (softmax rows sum to 1),
    # so its c_proj image is folded into the host-side bias add
    b_eff = b_proj + b_attn[2 * C :] @ w_proj.T
    out = np.empty((B, T, C), np.float32)
    for b in range(B):
        out[b] = res.results[2 * b]["out"] + res.results[2 * b + 1]["out"] + b_eff
    return out



# revision 42
# speedup vs baseline: 1.3624x; 1.0217x over previous
"""Causal self-attention (B=4, T=2048, C=1024, H=16) on 8 NeuronCores.

Sharding: core = (batch b, head-group g): data-parallel over B=4, tensor-
parallel over heads (2 groups x 8 heads).  Each core computes QKV + attention
for its 8 heads and the matching half of the c_proj contraction; the host
sums the two partial c_proj outputs per batch and adds b_proj.

Device layout notes:
  - all matmul operands bf16 (PE runs fp32 at 1/4 rate), PSUM f32
  - x, weights are pre-transposed on the host so every matmul contraction
    sits on the partition dim; no on-device transposes anywhere
  - QKV biases enter as K=1 rank-1 matmuls against a ones row
  - S is computed transposed ([keys, queries]); exp(S/8) on ScalarE with no
    max-subtraction (logits bounded ~+-4 for this problem's scale)
  - causality at tile granularity: k-tiles above the diagonal are skipped,
    diagonal tiles multiplied by precomputed 0/1 masks after exp; diagonal
    tiles further restrict S/exp/PV to their valid column range
  - softmax denominator = ones column appended to each head's V; PV matmul
    emits [y.T | denom] per (head, q-chunk)

Scheduling notes (the performance-critical part):
  - ScalarE exp is the pacer (~0.83ns/col + ~190ns/instr).  S tiles for two
    consecutive k-tiles are written into one [128,1024] 2-bank PSUM tile and
    exp'd with ONE activation for non-diagonal pairs (amortizes the fixed
    overhead); diagonal pairs keep two narrow column-sliced activations.
  - one continuous S->exp->PV pipeline per head across ALL q-chunks (PSUM
    yps double-buffered) so there is no PV-drain bubble at chunk boundaries;
    PV emission lags S by 2 groups (3 for diagonal groups, hiding the GPSIMD
    mask-multiply latency).
  - input DMAs are issued biases -> (wqk[k], xt[k]) pairs -> wv -> masks ->
    wp, and the pair-0 QKV projection is emitted k-innermost so the PE
    consumes tiles as they land; attention starts right after the pair-0
    Q/K projection + first 4 V tiles instead of after the full pre-phase.
  - remaining QKV projection work (pair 1-3 Q/K, V tiles) is drip-fed into
    the attention stream as PE "filler" (keeps the PE dense so the HAM
    activity monitor does not clock-throttle it to 1.2 GHz), with
    deadline-ordered tags force-flushed (ensure) just before a consumer.
  - normalization is per (head, q-chunk): reciprocal of the denominator row
    straight off the copied ysb tile, broadcast to 64 partitions via a
    rank-1 f32r matmul, multiply on DVE; the PE-side broadcast matmul is
    delayed one group so it never waits on the DVE reciprocal.
  - c_proj is emitted per q-chunk of the LAST head as filler work (all
    other pairs' y are normalized long before), so the output projection
    and its DMA overlap the tail of attention instead of serializing.
"""

import os

import numpy as np
import ml_dtypes

B, T, C, H = 4, 2048, 1024, 16
D = 64          # head dim
HL = 8          # heads per core
CL = HL * D     # 512 local channels
TQ = 512        # query chunk (matmul moving dim)
TK = 128        # key tile (psum partition dim)
NQC = T // TQ   # 4 query chunks
NKT = T // TK   # 16 key tiles
VW = HL * (D + 1)  # 520: V with per-head ones column

_prog = None
last_results = None  # BassKernelResults of the most recent run (for test.py)


def _build_program():
    import concourse.mybir as mybir
    import concourse.tile as tile
    from concourse import bacc

    f32 = mybir.dt.float32
    f32r = mybir.dt.float32r
    bf16 = mybir.dt.bfloat16
    EXP = mybir.ActivationFunctionType.Exp

    nc = bacc.Bacc("TRN2", target_bir_lowering=False, debug=False)

    xt_d = nc.dram_tensor("xt", [8, 128, T], bf16, kind="ExternalInput")
    # pair-interleaved: cols 256g..256g+128 = Q pair g, +128..256 = K pair g
    wqk_d = nc.dram_tensor("wqk", [8, 128, 2 * CL], bf16, kind="ExternalInput")
    wv_d = nc.dram_tensor("wv", [8, 128, CL], bf16, kind="ExternalInput")
    # per-pair bias columns, f32: col 2*g = Q pair g, col 2*g+1 = K pair g
    bqk_d = nc.dram_tensor("bqk", [128, 8], f32, kind="ExternalInput")
    wp_d = nc.dram_tensor("wp", [4, 128, C], bf16, kind="ExternalInput")
    mask_d = nc.dram_tensor("mask", [4, 128, TQ], bf16, kind="ExternalInput")
    out_d = nc.dram_tensor("out", [T, C], f32, kind="ExternalOutput")

    with tile.TileContext(nc) as tc:
        with (
            tc.tile_pool(name="persist", bufs=1) as pp,
            tc.tile_pool(name="ptpool", bufs=6) as ptp,
            tc.tile_pool(name="stage", bufs=4) as sp,
            tc.tile_pool(name="small", bufs=3) as smp,
            tc.tile_pool(name="psA", bufs=2, space="PSUM") as psA,   # [128,1024] x2 = 4 banks
            tc.tile_pool(name="psF", bufs=2, space="PSUM") as psF,   # [128,512]  x2 = 2 banks
            tc.tile_pool(name="psY", bufs=2, space="PSUM") as psY,   # [128,512]  x2 = 2 banks
        ):
            # ---- persistent SBUF tensors ----
            xt = [pp.tile([128, T], bf16, name=f"xt{k}") for k in range(8)]
            wqk = [pp.tile([128, 2 * CL], bf16, name=f"wqk{k}") for k in range(8)]
            wv = [pp.tile([128, CL], bf16, name=f"wv{k}") for k in range(8)]
            wp = [pp.tile([128, C], bf16, name=f"wp{k}") for k in range(4)]
            maskt = [pp.tile([128, TQ], bf16, name=f"mask{j}") for j in range(4)]
            bqk_cols = pp.tile([128, 8], f32, name="bqk_cols")
            # two-hot selector matrices: one matmul broadcasts rec rows 64*i
            # and 64*i+32 of a [97,512] tile into partitions 0-63 / 64-127 of
            # a [128,512] bc tile, normalizing a whole head PAIR at once
            # (SBUF APs may only start at partition 0/32/64/96, hence the
            # 32-pitch of the rec rows)
            sel2 = [pp.tile([97, 128], bf16, name=f"sel2_{i}") for i in range(2)]

            # DMA issue order = deadline order; the tile framework makes each
            # consumer wait only for its own slice, so compute overlaps the
            # load.  The pair-0/j-0 slices the pre-phase consumes are issued
            # as small leading chunks, split across the TWO hwdge issue
            # engines (SP + ACT run their ~600ns-per-dma issue streams in
            # parallel), so the first matmul starts ~5us earlier than a
            # whole-tile wait.
            nc.scalar.dma_start(out=bqk_cols[:], in_=bqk_d[:])
            for k in range(8):
                nc.scalar.dma_start(out=xt[k][:, 0:TQ], in_=xt_d[k][:, 0:TQ])
            for k in range(4):
                nc.scalar.dma_start(out=wv[k][:], in_=wv_d[k])
            for k in range(8):
                nc.sync.dma_start(out=wqk[k][:, 0:256], in_=wqk_d[k][:, 0:256])
            for k in range(4, 8):
                nc.sync.dma_start(out=wv[k][:], in_=wv_d[k])
            for j in range(4):
                nc.sync.dma_start(out=maskt[j][:], in_=mask_d[j])
            for k in range(8):
                nc.sync.dma_start(
                    out=wqk[k][:, 256:], in_=wqk_d[k][:, 256:]
                )
            for k in range(8):
                nc.sync.dma_start(out=xt[k][:, TQ : 2 * TQ],
                                  in_=xt_d[k][:, TQ : 2 * TQ])
            for k in range(4):
                nc.sync.dma_start(out=wp[k][:], in_=wp_d[k])
            for k in range(8):
                nc.sync.dma_start(out=xt[k][:, 2 * TQ :],
                                  in_=xt_d[k][:, 2 * TQ :])
            for i in range(2):
                nc.vector.memset(sel2[i][:], 0.0)
                nc.vector.memset(sel2[i][64 * i : 64 * i + 1, 0:64], 1.0)
                nc.vector.memset(sel2[i][64 * i + 32 : 64 * i + 33, 64:128],
                                 1.0)

            # QT/KT in [channel, t] layout; channel tile g = head pair g
            qt = [pp.tile([128, T], bf16, name=f"qt{i}") for i in range(4)]
            kt = [pp.tile([128, T], bf16, name=f"kt{i}") for i in range(4)]
            # V in natural [t, channel] layout with a ones column per head
            vsb = [pp.tile([128, VW], bf16, name=f"v{i}") for i in range(NKT)]
            yt = [pp.tile([128, T], bf16, name=f"yt{i}") for i in range(4)]
            # softmax-denominator ones columns: written once here, V copies
            # below use a strided AP that skips them
            for it in range(NKT):
                v3 = vsb[it].rearrange("p (h c) -> p h c", h=HL)
                nc.vector.memset(v3[:, :, D : D + 1], 1.0)

            # ---- pair-0 Q/K j0 projection, k-innermost across the two halves
            # of one wide psA tile, so the PE consumes (wqk[k], xt[k]) as
            # they land from HBM
            def qk_wave(g, j0):
                slot = psA.tile([128, 2 * TQ], f32, name="ps_qk", tag="s_w")
                chs = [(qt, 0, j0), (kt, 1, j0)]
                for k in range(8):
                    for ci, (dst, qk, j) in enumerate(chs):
                        ps = slot[:, ci * TQ : (ci + 1) * TQ]
                        w0 = 256 * g + 128 * qk
                        nc.tensor.matmul(
                            ps,
                            lhsT=wqk[k][:, w0 : w0 + 128],
                            rhs=xt[k][:, j * TQ : (j + 1) * TQ],
                            start=(k == 0),
                            stop=(k == 7),
                        )
                for ci, (dst, qk, j) in enumerate(chs):
                    ps = slot[:, ci * TQ : (ci + 1) * TQ]
                    nc.vector.tensor_scalar_add(
                        dst[g][:, j * TQ : (j + 1) * TQ],
                        ps,
                        bqk_cols[:, 2 * g + qk : 2 * g + qk + 1],
                    )

            def emit_qk_filler(g):
                """One head-pair's Q.T and K.T projection as single-matmul
                closures, tagged per (pair, j) for deadline ensure()."""
                steps = []
                for j in range(NQC):
                    for dst, qk in ((qt, 0), (kt, 1)):
                        ph = {}

                        def step(k, ph=ph, dst=dst, qk=qk, j=j, g=g):
                            if k == 0:
                                ph["ps"] = psF.tile(
                                    [128, TQ], f32, name="ps_f", tag="fill"
                                )
                            if k < 8:
                                w0 = 256 * g + 128 * qk
                                nc.tensor.matmul(
                                    ph["ps"][:],
                                    lhsT=wqk[k][:, w0 : w0 + 128],
                                    rhs=xt[k][:, j * TQ : (j + 1) * TQ],
                                    start=(k == 0),
                                    stop=(k == 7),
                                )
                            else:
                                nc.vector.tensor_scalar_add(
                                    dst[g][:, j * TQ : (j + 1) * TQ],
                                    ph["ps"],
                                    bqk_cols[:, 2 * g + qk : 2 * g + qk + 1],
                                )

                        for k in range(9):
                            steps.append((f"qk{g}j{j}", lambda k=k, step=step: step(k)))
                return steps

            def v_chain_steps(it, h2):
                """V projection for 4 heads of one 128-row t-tile.  The copy
                out uses a 65-pitch strided AP that skips the preset ones
                columns (v bias is folded into b_proj on the host)."""
                ph = {}

                def step(k, ph=ph, it=it, h2=h2):
                    if k == 0:
                        ph["ps"] = psF.tile([128, TQ], f32, name="ps_v", tag="fill")
                    if k < 8:
                        nc.tensor.matmul(
                            ph["ps"][:, : CL // 2],
                            lhsT=xt[k][:, it * 128 : (it + 1) * 128],
                            rhs=wv[k][:, h2 * (CL // 2) : (h2 + 1) * (CL // 2)],
                            start=(k == 0),
                            stop=(k == 7),
                        )
                    else:
                        v3 = vsb[it].rearrange("p (h c) -> p h c", h=HL)
                        ps3 = ph["ps"].rearrange("p (h c) -> p h c", h=HL)
                        nc.vector.tensor_copy(
                            out=v3[:, 4 * h2 : 4 * h2 + 4, 0:D],
                            in_=ps3[:, 0:4, 0:D],
                        )

                return [(f"v{h2}k{it}", lambda k=k, step=step: step(k))
                        for k in range(9)]

            # pre-phase (inline, DMA-paced): pair-0 Q/K chunk j0 + the first
            # 4 V tiles of half 0 — the minimum sweep 0's first PVs consume
            qk_wave(0, 0)
            for it in range(4):
                for _, s in v_chain_steps(it, 0):
                    s()

            # ---- filler queue, deadline-ordered for the qc-major sweeps:
            # sweep s consumes chunk j=s of every pair and V t-tiles 4s..4s+3
            fillers = []
            qk_steps = {g: emit_qk_filler(g) for g in range(4)}
            for g in (1, 2):
                fillers.extend(qk_steps[g][0:18])         # qk{1,2}j0, due h2/h4
            for it in range(4):                           # V half1 0-3, due h4
                fillers.extend(v_chain_steps(it, 1))
            fillers.extend(qk_steps[3][0:18])             # qk3j0, due h6
            for s in range(1, NQC):
                for g in range(4):
                    fillers.extend(qk_steps[g][s * 18 : (s + 1) * 18])
                for it in range(4 * s, 4 * s + 4):
                    for h2 in range(2):
                        fillers.extend(v_chain_steps(it, h2))

            def drain(n):
                for _ in range(n):
                    if fillers:
                        fillers.pop(0)[1]()

            def ensure(tag):
                """Force-emit exactly the closures of `tag` (chains of
                different tags are mutually independent, so skipping others
                is safe and avoids serializing the whole backlog)."""
                keep, run = [], []
                for t, s in fillers:
                    (run if t == tag else keep).append((t, s))
                fillers[:] = keep
                for _, s in run:
                    s()

            # ---- attention: one continuous S->exp->PV stream per head ----
            def c_proj_chain(it, oc):
                """c_proj for one [128,512] output tile: 4-matmul chain +
                copy + DMA, as single-step closures."""
                ph = {}

                def step(s, ph=ph, it=it, oc=oc):
                    if s < 4:
                        if s == 0:
                            ph["ps"] = psF.tile([128, TQ], f32, name="ps_o",
                                                tag="fill")
                        nc.tensor.matmul(
                            ph["ps"][:],
                            lhsT=yt[s][:, it * 128 : (it + 1) * 128],
                            rhs=wp[s][:, oc * TQ : (oc + 1) * TQ],
                            start=(s == 0),
                            stop=(s == 3),
                        )
                    elif s == 4:
                        ph["ot"] = sp.tile([128, TQ], f32, name="ot")
                        nc.vector.tensor_copy(out=ph["ot"][:], in_=ph["ps"][:])
                    else:
                        nc.sync.dma_start(
                            out=out_d[it * 128 : (it + 1) * 128,
                                      oc * TQ : (oc + 1) * TQ],
                            in_=ph["ot"][:],
                        )

                return [("cp", lambda s=s, step=step: step(s)) for s in range(6)]

            # qc-major sweeps: process q-chunk qc for ALL 8 heads, then qc+1.
            # c_proj for a t-range starts one sweep after its columns are
            # final, so the output projection + DMA overlap attention instead
            # of serializing after it.
            pend = []     # (h, qc, g, diag) whose PV is not yet emitted
            yps = {}      # h -> current PSUM y accumulator [65, TQ]
            pts = {}      # (h, qc, g) -> pt tile
            sweep_state = {}  # qc -> (dens[2], slots[2])
            pair_cur = {}  # grp -> pair ysb tile being filled
            tail_reserve = []  # c_proj chains held for the final norm window

            def norm_steps(grp, qc):
                """Normalization of 4 heads: one full-tile approx reciprocal
                of the batched den tile (rows filled per head in post_pv,
                straight off the PSUM denominator row), one f32->bf16 cast,
                then per-head broadcast+multiply closures.  bf16 rec keeps
                the rank-1 broadcast matmul at full bf16 rate."""
                dens, slots = sweep_state[qc]
                den_g, group = dens[grp], slots[grp]
                rec_g = smp.tile([97, TQ], f32, name="rec_g", tag="recg",
                                 bufs=3)
                rec_b = smp.tile([97, TQ], bf16, name="rec_b", tag="recb",
                                 bufs=3)
                steps = [
                    lambda: nc.vector.reciprocal_approx_fast(
                        out=rec_g[:], in_=den_g[:]),
                    lambda: nc.vector.tensor_copy(out=rec_b[:], in_=rec_g[:]),
                ]

                def one(p, ysb2, qc=qc, rec_b=rec_b, grp=grp):
                    bc = psF.tile([128, TQ], f32, name="bc", tag="fill")
                    nc.tensor.matmul(
                        bc[:], lhsT=sel2[p][:], rhs=rec_b[:],
                        start=True, stop=True,
                    )
                    nc.vector.tensor_mul(
                        yt[2 * grp + p][:, qc * TQ : (qc + 1) * TQ],
                        ysb2[:],
                        bc[:],
                    )

                for p, ysb2 in group:
                    steps.append(lambda p=p, ysb2=ysb2: one(p, ysb2))
                return steps

            def c_proj_tail(qc, norm_tail):
                """Final sweep's c_proj on wide PSUM (the S pipeline is done
                with psA by now).  Pairs 0/1 of the contraction only need
                group-0 heads (normalized long ago), so two tiles' worth of
                those matmuls are emitted first and the final group's
                broadcast+multiply steps (`norm_tail`) are woven between
                them to hide the normalization round-trip latency."""
                slots = {}

                def front_half(p):
                    it = 4 * qc + p
                    slots[p] = psA.tile([128, 2 * TQ], f32, name="ps_cp",
                                        tag="s_w")
                    for ic in range(2):
                        for oc in range(2):
                            nc.tensor.matmul(
                                slots[p][:, oc * TQ : (oc + 1) * TQ],
                                lhsT=yt[ic][:, it * 128 : (it + 1) * 128],
                                rhs=wp[ic][:, oc * TQ : (oc + 1) * TQ],
                                start=(ic == 0),
                                stop=False,
                            )

                def back_half(p):
                    it = 4 * qc + p
                    for ic in range(2, 4):
                        for oc in range(2):
                            nc.tensor.matmul(
                                slots[p][:, oc * TQ : (oc + 1) * TQ],
                                lhsT=yt[ic][:, it * 128 : (it + 1) * 128],
                                rhs=wp[ic][:, oc * TQ : (oc + 1) * TQ],
                                start=False,
                                stop=(ic == 3),
                            )
                    ot = sp.tile([128, 2 * TQ], f32, name="otw", tag="otw",
                                 bufs=2)
                    for oc in range(2):
                        nc.vector.tensor_copy(
                            out=ot[:, oc * TQ : (oc + 1) * TQ],
                            in_=slots[p][:, oc * TQ : (oc + 1) * TQ],
                        )
                        nc.sync.dma_start(
                            out=out_d[it * 128 : (it + 1) * 128,
                                      oc * TQ : (oc + 1) * TQ],
                            in_=ot[:, oc * TQ : (oc + 1) * TQ],
                        )

                norm_tail = list(norm_tail)
                front_half(0)
                if norm_tail:
                    norm_tail.pop(0)()          # bcast+mul heads 4,5
                front_half(1)
                while norm_tail:
                    norm_tail.pop(0)()          # bcast+mul heads 6,7
                for p in range(2):
                    back_half(p)
                for p in range(2, 4):
                    front_half(p)
                    back_half(p)

            def post_pv(h, qc):
                """After the last PV of (h, qc): copy the PSUM denominator
                row into this head's row of the group's batched den tile,
                then copy the head's y into its half of the PAIR's packed
                [128,TQ] ysb tile (so normalization runs one full-width
                bcast+multiply per head pair)."""
                dens, slots = sweep_state[qc]
                yp = yps.pop(h)
                r0 = 32 * (h % 4)
                nc.vector.tensor_copy(
                    out=dens[h // 4][r0 : r0 + 1, :], in_=yp[64:65, :]
                )
                if h % 2 == 0:
                    pair_cur[h // 4] = smp.tile([128, TQ], f32, name="ysb2",
                                                tag="ysb", bufs=6)
                po = 64 * (h % 2)
                nc.vector.tensor_copy(
                    out=pair_cur[h // 4][po : po + 64, :], in_=yp[0:64, :]
                )
                if h % 2 == 1:
                    slots[h // 4].append(
                        ((h % 4) // 2, pair_cur.pop(h // 4))
                    )
                final = h == HL - 1 and qc == NQC - 1
                if h % 4 == 3 and not final:
                    fillers.extend(("norm", s) for s in norm_steps(h // 4, qc))
                if h == HL - 1:
                    if final:
                        # cover the last head's reciprocal latency with the
                        # reserved c_proj chains so the PE stays hot, then
                        # weave group 1's normalization into the first tail
                        # c_proj chains (whose pair-0/1 contractions only
                        # need the long-finished group-0 heads)
                        steps = norm_steps(1, qc)
                        steps[0](), steps[1]()      # recip + cast
                        for _, s in tail_reserve:
                            s()
                        tail_reserve.clear()
                        drain(8)
                        c_proj_tail(qc, steps[2:])
                    else:
                        for it in range(4 * qc, 4 * qc + 4):
                            for oc in range(2):
                                steps = c_proj_chain(it, oc)
                                if qc == NQC - 2 and it >= 4 * qc + 2:
                                    tail_reserve.extend(steps)
                                else:
                                    fillers.extend(steps)
                    del sweep_state[qc]

            def pv_group(h, qc, g):
                ktop = (qc + 1) * (TQ // TK)
                pt_w = pts.pop((h, qc, g))
                for hh in range(2):
                    ensure(f"v{h // 4}k{2 * g + hh}")
                for hh in range(2):
                    ktl = 2 * g + hh
                    j = ktl - qc * (TQ // TK)
                    col0 = j * TK if j >= 0 else 0
                    if ktl == 0:
                        yps[h] = psY.tile([D + 1, TQ], f32, name="yps",
                                          tag="y")
                    nc.tensor.matmul(
                        yps[h][:, col0:],
                        lhsT=vsb[ktl][:, h * 65 : (h + 1) * 65],
                        rhs=pt_w[:, hh * TQ + col0 : (hh + 1) * TQ],
                        start=(ktl == 0),
                        stop=(ktl == ktop - 1),
                    )
                if 2 * g + 1 == ktop - 1:
                    post_pv(h, qc)

            def s_group(h, qc, g):
                g2, po = h // 2, 64 * (h % 2)
                diag = 2 * g >= 4 * qc
                ps_s = psA.tile([128, 2 * TQ], f32, name="ps_s", tag="s_w")
                pt_w = ptp.tile([128, 2 * TQ], bf16, name="pt")
                for hh in range(2):
                    ktl = 2 * g + hh
                    j = ktl - qc * (TQ // TK)
                    col0 = j * TK if j >= 0 else 0
                    nc.tensor.matmul(
                        ps_s[:, hh * TQ + col0 : (hh + 1) * TQ],
                        lhsT=kt[g2][po : po + 64, ktl * TK : (ktl + 1) * TK],
                        rhs=qt[g2][po : po + 64,
                                   qc * TQ + col0 : (qc + 1) * TQ],
                        start=True,
                        stop=True,
                    )
                if not diag:
                    # one wide exp over both k-tiles (2 PSUM banks)
                    nc.scalar.activation(pt_w[:, :], ps_s[:, :], EXP,
                                         scale=0.125)
                else:
                    for hh in range(2):
                        ktl = 2 * g + hh
                        j = ktl - qc * (TQ // TK)
                        col0 = j * TK
                        nc.scalar.activation(
                            pt_w[:, hh * TQ + col0 : (hh + 1) * TQ],
                            ps_s[:, hh * TQ + col0 : (hh + 1) * TQ],
                            EXP,
                            scale=0.125,
                        )
                        nc.gpsimd.tensor_mul(
                            pt_w[:, hh * TQ + col0 : (hh + 1) * TQ],
                            pt_w[:, hh * TQ + col0 : (hh + 1) * TQ],
                            maskt[j][:, col0:],
                        )
                pts[(h, qc, g)] = pt_w
                return diag

            def tick():
                """After each S group emission: drip fillers, emit lagged PV
                groups (deeper lag for diagonal groups hides mask latency)."""
                drain(4 if len(fillers) > 160 else 3 if len(fillers) > 40
                      else 2)
                if pend and len(pend) >= (3 if pend[0][3] else 2):
                    h0, qc0, g0, _ = pend.pop(0)
                    pv_group(h0, qc0, g0)

            for qc in range(NQC):
                # flush previous sweeps' normalization before re-using the
                # rec/ysb slots (keeps the in-order DVE queue acyclic)
                ensure("norm")
                dens = [smp.tile([97, TQ], f32, name=f"den{i}", tag="deng",
                                 bufs=4) for i in range(2)]
                for dg in dens:
                    nc.vector.memset(dg[:], 1.0)  # rows between heads unused
                sweep_state[qc] = (dens, [[], []])
                for h in range(HL):
                    g2 = h // 2
                    ensure(f"qk{g2}j{qc}")
                    # spread V-tile prefetch over the heads whose PVs lag
                    if h in (0, 1, 4, 5):
                        half, o = h // 4, 2 * (h % 4)
                        ensure(f"v{half}k{4 * qc + o}")
                        ensure(f"v{half}k{4 * qc + o + 1}")
                    # prefetch next sweep's projection chunks while this
                    # sweep's exp stream can still hide the PE work
                    if qc < NQC - 1 and h >= 4:
                        ensure(f"qk{h - 4}j{qc + 1}")
                    for g in range((qc + 1) * (TQ // TK) // 2):
                        diag = s_group(h, qc, g)
                        pend.append((h, qc, g, diag))
                        tick()
            while pend:
                h0, qc0, g0, _ = pend.pop(0)
                pv_group(h0, qc0, g0)
                drain(1)

            drain(len(fillers))

    nc.finalize()
    return nc


def _bf16(a):
    return np.ascontiguousarray(a, dtype=np.float32).astype(ml_dtypes.bfloat16)


def _core_inputs(x, w_attn, b_attn, w_proj, masks, core):
    b, g = divmod(core, 2)
    gs = slice(g * CL, (g + 1) * CL)
    wq, wk, wv_ = (w_attn[i * C : (i + 1) * C][gs] for i in range(3))
    bq, bk = (b_attn[i * C : (i + 1) * C][gs] for i in range(2))

    # pair-interleaved QKV weight: col block 2p = Q pair p, 2p+1 = K pair p
    wqkT = np.empty((C, 2 * CL), np.float32)
    for p in range(4):
        wqkT[:, 256 * p : 256 * p + 128] = wq.T[:, 128 * p : 128 * (p + 1)]
        wqkT[:, 256 * p + 128 : 256 * (p + 1)] = wk.T[:, 128 * p : 128 * (p + 1)]
    # f32 bias columns, col 2p = Q pair p, col 2p+1 = K pair p
    bqk_cols = np.empty((128, 8), np.float32)
    for p in range(4):
        bqk_cols[:, 2 * p] = bq[128 * p : 128 * (p + 1)]
        bqk_cols[:, 2 * p + 1] = bk[128 * p : 128 * (p + 1)]

    return {
        "xt": _bf16(x[b].T).reshape(8, 128, T),
        "wqk": _bf16(wqkT).reshape(8, 128, 2 * CL),
        "wv": _bf16(wv_.T).reshape(8, 128, CL),
        "bqk": bqk_cols,
        "wp": _bf16(w_proj[:, gs].T).reshape(4, 128, C),
        "mask": masks,
    }


def _make_masks():
    qq = np.arange(TQ)[None, :]
    kk = np.arange(TK)[:, None]
    m = np.stack([(qq >= kk + j * TK) for j in range(4)]).astype(np.float32)
    return m.astype(ml_dtypes.bfloat16)


def kernel(x, w_attn, b_attn, w_proj, b_proj):
    global _prog, last_results
    from concourse.bass_utils import run_bass_kernel_spmd

    if _prog is None:
        _prog = _build_program()

    x = np.asarray(x, np.float32)
    w_attn = np.asarray(w_attn, np.float32)
    b_attn = np.asarray(b_attn, np.float32)
    w_proj = np.asarray(w_proj, np.float32)
    b_proj = np.asarray(b_proj, np.float32)

    masks = _make_masks()
    in_maps = [
        _core_inputs(x, w_attn, b_attn, w_proj, masks, core) for core in range(8)
    ]
    kwargs = {}
    tmpdir = os.environ.get("BASS_TMPDIR")
    if tmpdir:
        os.makedirs(tmpdir, exist_ok=True)
        kwargs["tmpdir"] = tmpdir
    res = run_bass_kernel_spmd(_prog, in_maps, list(range(8)), **kwargs)
    last_results = res

    # v-bias passes through attention as a constant (softmax rows sum to 1),
    # so its c_proj image is folded into the host-side bias add
    b_eff = b_proj + b_attn[2 * C :] @ w_proj.T
    out = np.empty((B, T, C), np.float32)
    for b in range(B):
        out[b] = res.results[2 * b]["out"] + res.results[2 * b + 1]["out"] + b_eff
    return out



# revision 44
# speedup vs baseline: 1.3704x; 1.0059x over previous
"""Causal self-attention (B=4, T=2048, C=1024, H=16) on 8 NeuronCores.

Sharding: core = (batch b, head-group g): data-parallel over B=4, tensor-
parallel over heads (2 groups x 8 heads).  Each core computes QKV + attention
for its 8 heads and the matching half of the c_proj contraction; the host
sums the two partial c_proj outputs per batch and adds b_proj.

Device layout notes:
  - all matmul operands bf16 (PE runs fp32 at 1/4 rate), PSUM f32
  - x, weights are pre-transposed on the host so every matmul contraction
    sits on the partition dim; no on-device transposes anywhere
  - QKV biases enter as K=1 rank-1 matmuls against a ones row
  - S is computed transposed ([keys, queries]); exp(S/8) on ScalarE with no
    max-subtraction (logits bounded ~+-4 for this problem's scale)
  - causality at tile granularity: k-tiles above the diagonal are skipped,
    diagonal tiles multiplied by precomputed 0/1 masks after exp; diagonal
    tiles further restrict S/exp/PV to their valid column range
  - softmax denominator = ones column appended to each head's V; PV matmul
    emits [y.T | denom] per (head, q-chunk)

Scheduling notes (the performance-critical part):
  - ScalarE exp is the pacer (~0.83ns/col + ~190ns/instr).  S tiles for two
    consecutive k-tiles are written into one [128,1024] 2-bank PSUM tile and
    exp'd with ONE activation for non-diagonal pairs (amortizes the fixed
    overhead); diagonal pairs keep two narrow column-sliced activations.
  - one continuous S->exp->PV pipeline per head across ALL q-chunks (PSUM
    yps double-buffered) so there is no PV-drain bubble at chunk boundaries;
    PV emission lags S by 2 groups (3 for diagonal groups, hiding the GPSIMD
    mask-multiply latency).
  - input DMAs are issued biases -> (wqk[k], xt[k]) pairs -> wv -> masks ->
    wp, and the pair-0 QKV projection is emitted k-innermost so the PE
    consumes tiles as they land; attention starts right after the pair-0
    Q/K projection + first 4 V tiles instead of after the full pre-phase.
  - remaining QKV projection work (pair 1-3 Q/K, V tiles) is drip-fed into
    the attention stream as PE "filler" (keeps the PE dense so the HAM
    activity monitor does not clock-throttle it to 1.2 GHz), with
    deadline-ordered tags force-flushed (ensure) just before a consumer.
  - normalization is per (head, q-chunk): reciprocal of the denominator row
    straight off the copied ysb tile, broadcast to 64 partitions via a
    rank-1 f32r matmul, multiply on DVE; the PE-side broadcast matmul is
    delayed one group so it never waits on the DVE reciprocal.
  - c_proj is emitted per q-chunk of the LAST head as filler work (all
    other pairs' y are normalized long before), so the output projection
    and its DMA overlap the tail of attention instead of serializing.
"""

import os

import numpy as np
import ml_dtypes

B, T, C, H = 4, 2048, 1024, 16
D = 64          # head dim
HL = 8          # heads per core
CL = HL * D     # 512 local channels
TQ = 512        # query chunk (matmul moving dim)
TK = 128        # key tile (psum partition dim)
NQC = T // TQ   # 4 query chunks
NKT = T // TK   # 16 key tiles
VW = HL * (D + 1)  # 520: V with per-head ones column

_prog = None
last_results = None  # BassKernelResults of the most recent run (for test.py)


def _build_program():
    import concourse.mybir as mybir
    import concourse.tile as tile
    from concourse import bacc

    f32 = mybir.dt.float32
    f32r = mybir.dt.float32r
    bf16 = mybir.dt.bfloat16
    EXP = mybir.ActivationFunctionType.Exp

    nc = bacc.Bacc("TRN2", target_bir_lowering=False, debug=False)

    xt_d = nc.dram_tensor("xt", [8, 128, T], bf16, kind="ExternalInput")
    # pair-interleaved: cols 256g..256g+128 = Q pair g, +128..256 = K pair g
    wqk_d = nc.dram_tensor("wqk", [8, 128, 2 * CL], bf16, kind="ExternalInput")
    wv_d = nc.dram_tensor("wv", [8, 128, CL], bf16, kind="ExternalInput")
    # per-pair bias columns, f32: col 2*g = Q pair g, col 2*g+1 = K pair g
    bqk_d = nc.dram_tensor("bqk", [128, 8], f32, kind="ExternalInput")
    wp_d = nc.dram_tensor("wp", [4, 128, C], bf16, kind="ExternalInput")
    mask_d = nc.dram_tensor("mask", [4, 128, TQ], bf16, kind="ExternalInput")
    out_d = nc.dram_tensor("out", [T, C], f32, kind="ExternalOutput")

    with tile.TileContext(nc) as tc:
        with (
            tc.tile_pool(name="persist", bufs=1) as pp,
            tc.tile_pool(name="ptpool", bufs=6) as ptp,
            tc.tile_pool(name="stage", bufs=4) as sp,
            tc.tile_pool(name="small", bufs=3) as smp,
            tc.tile_pool(name="psA", bufs=2, space="PSUM") as psA,   # [128,1024] x2 = 4 banks
            tc.tile_pool(name="psF", bufs=2, space="PSUM") as psF,   # [128,512]  x2 = 2 banks
            tc.tile_pool(name="psY", bufs=2, space="PSUM") as psY,   # [128,512]  x2 = 2 banks
        ):
            # ---- persistent SBUF tensors ----
            xt = [pp.tile([128, T], bf16, name=f"xt{k}") for k in range(8)]
            wqk = [pp.tile([128, 2 * CL], bf16, name=f"wqk{k}") for k in range(8)]
            wv = [pp.tile([128, CL], bf16, name=f"wv{k}") for k in range(8)]
            wp = [pp.tile([128, C], bf16, name=f"wp{k}") for k in range(4)]
            maskt = [pp.tile([128, TQ], bf16, name=f"mask{j}") for j in range(4)]
            bqk_cols = pp.tile([128, 8], f32, name="bqk_cols")
            # two-hot selector matrices: one matmul broadcasts rec rows 64*i
            # and 64*i+32 of a [97,512] tile into partitions 0-63 / 64-127 of
            # a [128,512] bc tile, normalizing a whole head PAIR at once
            # (SBUF APs may only start at partition 0/32/64/96, hence the
            # 32-pitch of the rec rows)
            sel2 = [pp.tile([97, 128], bf16, name=f"sel2_{i}") for i in range(2)]

            # DMA issue order = deadline order; the tile framework makes each
            # consumer wait only for its own slice, so compute overlaps the
            # load.  The pair-0/j-0 slices the pre-phase consumes are issued
            # as small leading chunks, split across the TWO hwdge issue
            # engines (SP + ACT run their ~600ns-per-dma issue streams in
            # parallel), so the first matmul starts ~5us earlier than a
            # whole-tile wait.
            for k in range(8):
                nc.scalar.dma_start(out=xt[k][:, 0:TQ], in_=xt_d[k][:, 0:TQ])
            nc.scalar.dma_start(out=bqk_cols[:], in_=bqk_d[:])
            for k in range(4):
                nc.scalar.dma_start(out=wv[k][:], in_=wv_d[k])
            for k in range(8):
                nc.sync.dma_start(out=wqk[k][:, 0:256], in_=wqk_d[k][:, 0:256])
            for k in range(4, 8):
                nc.sync.dma_start(out=wv[k][:], in_=wv_d[k])
            for j in range(4):
                nc.sync.dma_start(out=maskt[j][:], in_=mask_d[j])
            for k in range(8):
                nc.sync.dma_start(
                    out=wqk[k][:, 256:], in_=wqk_d[k][:, 256:]
                )
            for k in range(8):
                nc.sync.dma_start(out=xt[k][:, TQ : 2 * TQ],
                                  in_=xt_d[k][:, TQ : 2 * TQ])
            for k in range(4):
                nc.sync.dma_start(out=wp[k][:], in_=wp_d[k])
            for k in range(8):
                nc.sync.dma_start(out=xt[k][:, 2 * TQ :],
                                  in_=xt_d[k][:, 2 * TQ :])
            for i in range(2):
                nc.vector.memset(sel2[i][:], 0.0)
                nc.vector.memset(sel2[i][64 * i : 64 * i + 1, 0:64], 1.0)
                nc.vector.memset(sel2[i][64 * i + 32 : 64 * i + 33, 64:128],
                                 1.0)
            # warm-up: ~3.5us of dummy matmuls on a zeroed tile while the
            # first input DMAs land, so the HAM activity monitor has ramped
            # the PE to full clock (3us of continuous busy) before the real
            # instruction stream starts -- and stays there (no idle gap)
            warm = pp.tile([97, TQ], bf16, name="warm")
            nc.vector.memset(warm[:], 0.0)
            for _ in range(8):
                wps = psF.tile([128, TQ], f32, name="ps_warm", tag="fill")
                nc.tensor.matmul(wps[:], lhsT=warm[:, 0:128], rhs=warm[:],
                                 start=True, stop=True)

            # QT/KT in [channel, t] layout; channel tile g = head pair g
            qt = [pp.tile([128, T], bf16, name=f"qt{i}") for i in range(4)]
            kt = [pp.tile([128, T], bf16, name=f"kt{i}") for i in range(4)]
            # V in natural [t, channel] layout with a ones column per head
            vsb = [pp.tile([128, VW], bf16, name=f"v{i}") for i in range(NKT)]
            yt = [pp.tile([128, T], bf16, name=f"yt{i}") for i in range(4)]
            # softmax-denominator ones columns: written once here, V copies
            # below use a strided AP that skips them
            for it in range(NKT):
                v3 = vsb[it].rearrange("p (h c) -> p h c", h=HL)
                nc.vector.memset(v3[:, :, D : D + 1], 1.0)

            # ---- pair-0 Q/K j0 projection, k-innermost across the two halves
            # of one wide psA tile, so the PE consumes (wqk[k], xt[k]) as
            # they land from HBM
            def qk_wave(g, j0):
                slot = psA.tile([128, 2 * TQ], f32, name="ps_qk", tag="s_w")
                chs = [(qt, 0, j0), (kt, 1, j0)]
                for k in range(8):
                    for ci, (dst, qk, j) in enumerate(chs):
                        ps = slot[:, ci * TQ : (ci + 1) * TQ]
                        w0 = 256 * g + 128 * qk
                        nc.tensor.matmul(
                            ps,
                            lhsT=wqk[k][:, w0 : w0 + 128],
                            rhs=xt[k][:, j * TQ : (j + 1) * TQ],
                            start=(k == 0),
                            stop=(k == 7),
                        )
                for ci, (dst, qk, j) in enumerate(chs):
                    ps = slot[:, ci * TQ : (ci + 1) * TQ]
                    nc.vector.tensor_scalar_add(
                        dst[g][:, j * TQ : (j + 1) * TQ],
                        ps,
                        bqk_cols[:, 2 * g + qk : 2 * g + qk + 1],
                    )

            def emit_qk_filler(g):
                """One head-pair's Q.T and K.T projection as single-matmul
                closures, tagged per (pair, j) for deadline ensure()."""
                steps = []
                for j in range(NQC):
                    for dst, qk in ((qt, 0), (kt, 1)):
                        ph = {}

                        def step(k, ph=ph, dst=dst, qk=qk, j=j, g=g):
                            if k == 0:
                                ph["ps"] = psF.tile(
                                    [128, TQ], f32, name="ps_f", tag="fill"
                                )
                            if k < 8:
                                w0 = 256 * g + 128 * qk
                                nc.tensor.matmul(
                                    ph["ps"][:],
                                    lhsT=wqk[k][:, w0 : w0 + 128],
                                    rhs=xt[k][:, j * TQ : (j + 1) * TQ],
                                    start=(k == 0),
                                    stop=(k == 7),
                                )
                            else:
                                nc.vector.tensor_scalar_add(
                                    dst[g][:, j * TQ : (j + 1) * TQ],
                                    ph["ps"],
                                    bqk_cols[:, 2 * g + qk : 2 * g + qk + 1],
                                )

                        for k in range(9):
                            steps.append((f"qk{g}j{j}", lambda k=k, step=step: step(k)))
                return steps

            def v_chain_steps(it, h2):
                """V projection for 4 heads of one 128-row t-tile.  The copy
                out uses a 65-pitch strided AP that skips the preset ones
                columns (v bias is folded into b_proj on the host)."""
                ph = {}

                def step(k, ph=ph, it=it, h2=h2):
                    if k == 0:
                        ph["ps"] = psF.tile([128, TQ], f32, name="ps_v", tag="fill")
                    if k < 8:
                        nc.tensor.matmul(
                            ph["ps"][:, : CL // 2],
                            lhsT=xt[k][:, it * 128 : (it + 1) * 128],
                            rhs=wv[k][:, h2 * (CL // 2) : (h2 + 1) * (CL // 2)],
                            start=(k == 0),
                            stop=(k == 7),
                        )
                    else:
                        v3 = vsb[it].rearrange("p (h c) -> p h c", h=HL)
                        ps3 = ph["ps"].rearrange("p (h c) -> p h c", h=HL)
                        nc.vector.tensor_copy(
                            out=v3[:, 4 * h2 : 4 * h2 + 4, 0:D],
                            in_=ps3[:, 0:4, 0:D],
                        )

                return [(f"v{h2}k{it}", lambda k=k, step=step: step(k))
                        for k in range(9)]

            # pre-phase (inline, DMA-paced): pair-0 Q/K chunk j0 + the first
            # 4 V tiles of half 0 — the minimum sweep 0's first PVs consume
            qk_wave(0, 0)
            for it in range(4):
                for _, s in v_chain_steps(it, 0):
                    s()

            # ---- filler queue, deadline-ordered for the qc-major sweeps:
            # sweep s consumes chunk j=s of every pair and V t-tiles 4s..4s+3
            fillers = []
            qk_steps = {g: emit_qk_filler(g) for g in range(4)}
            for g in (1, 2):
                fillers.extend(qk_steps[g][0:18])         # qk{1,2}j0, due h2/h4
            for it in range(4):                           # V half1 0-3, due h4
                fillers.extend(v_chain_steps(it, 1))
            fillers.extend(qk_steps[3][0:18])             # qk3j0, due h6
            for s in range(1, NQC):
                for g in range(4):
                    fillers.extend(qk_steps[g][s * 18 : (s + 1) * 18])
                for it in range(4 * s, 4 * s + 4):
                    for h2 in range(2):
                        fillers.extend(v_chain_steps(it, h2))

            def drain(n):
                for _ in range(n):
                    if fillers:
                        fillers.pop(0)[1]()

            def ensure(tag):
                """Force-emit exactly the closures of `tag` (chains of
                different tags are mutually independent, so skipping others
                is safe and avoids serializing the whole backlog)."""
                keep, run = [], []
                for t, s in fillers:
                    (run if t == tag else keep).append((t, s))
                fillers[:] = keep
                for _, s in run:
                    s()

            # ---- attention: one continuous S->exp->PV stream per head ----
            def c_proj_chain(it, oc):
                """c_proj for one [128,512] output tile: 4-matmul chain +
                copy + DMA, as single-step closures."""
                ph = {}

                def step(s, ph=ph, it=it, oc=oc):
                    if s < 4:
                        if s == 0:
                            ph["ps"] = psF.tile([128, TQ], f32, name="ps_o",
                                                tag="fill")
                        nc.tensor.matmul(
                            ph["ps"][:],
                            lhsT=yt[s][:, it * 128 : (it + 1) * 128],
                            rhs=wp[s][:, oc * TQ : (oc + 1) * TQ],
                            start=(s == 0),
                            stop=(s == 3),
                        )
                    elif s == 4:
                        ph["ot"] = sp.tile([128, TQ], f32, name="ot")
                        nc.vector.tensor_copy(out=ph["ot"][:], in_=ph["ps"][:])
                    else:
                        nc.sync.dma_start(
                            out=out_d[it * 128 : (it + 1) * 128,
                                      oc * TQ : (oc + 1) * TQ],
                            in_=ph["ot"][:],
                        )

                return [("cp", lambda s=s, step=step: step(s)) for s in range(6)]

            # qc-major sweeps: process q-chunk qc for ALL 8 heads, then qc+1.
            # c_proj for a t-range starts one sweep after its columns are
            # final, so the output projection + DMA overlap attention instead
            # of serializing after it.
            pend = []     # (h, qc, g, diag) whose PV is not yet emitted
            yps = {}      # h -> current PSUM y accumulator [65, TQ]
            pts = {}      # (h, qc, g) -> pt tile
            sweep_state = {}  # qc -> (dens[2], slots[2])
            pair_cur = {}  # grp -> pair ysb tile being filled
            tail_reserve = []  # c_proj chains held for the final norm window

            def norm_steps(grp, qc):
                """Normalization of 4 heads: one full-tile approx reciprocal
                of the batched den tile (rows filled per head in post_pv,
                straight off the PSUM denominator row), one f32->bf16 cast,
                then per-head broadcast+multiply closures.  bf16 rec keeps
                the rank-1 broadcast matmul at full bf16 rate."""
                dens, slots = sweep_state[qc]
                den_g, group = dens[grp], slots[grp]
                rec_g = smp.tile([97, TQ], f32, name="rec_g", tag="recg",
                                 bufs=3)
                rec_b = smp.tile([97, TQ], bf16, name="rec_b", tag="recb",
                                 bufs=3)
                steps = [
                    lambda: nc.vector.reciprocal_approx_fast(
                        out=rec_g[:], in_=den_g[:]),
                    lambda: nc.vector.tensor_copy(out=rec_b[:], in_=rec_g[:]),
                ]

                def one(p, ysb2, qc=qc, rec_b=rec_b, grp=grp):
                    bc = psF.tile([128, TQ], f32, name="bc", tag="fill")
                    nc.tensor.matmul(
                        bc[:], lhsT=sel2[p][:], rhs=rec_b[:],
                        start=True, stop=True,
                    )
                    nc.vector.tensor_mul(
                        yt[2 * grp + p][:, qc * TQ : (qc + 1) * TQ],
                        ysb2[:],
                        bc[:],
                    )

                for p, ysb2 in group:
                    steps.append(lambda p=p, ysb2=ysb2: one(p, ysb2))
                return steps

            def c_proj_tail(qc, norm_tail):
                """Final sweep's c_proj on wide PSUM (the S pipeline is done
                with psA by now).  Pairs 0/1 of the contraction only need
                group-0 heads (normalized long ago), so two tiles' worth of
                those matmuls are emitted first and the final group's
                broadcast+multiply steps (`norm_tail`) are woven between
                them to hide the normalization round-trip latency."""
                slots = {}

                def front_half(p):
                    it = 4 * qc + p
                    slots[p] = psA.tile([128, 2 * TQ], f32, name="ps_cp",
                                        tag="s_w")
                    for ic in range(2):
                        for oc in range(2):
                            nc.tensor.matmul(
                                slots[p][:, oc * TQ : (oc + 1) * TQ],
                                lhsT=yt[ic][:, it * 128 : (it + 1) * 128],
                                rhs=wp[ic][:, oc * TQ : (oc + 1) * TQ],
                                start=(ic == 0),
                                stop=False,
                            )

                def back_half(p):
                    it = 4 * qc + p
                    for ic in range(2, 4):
                        for oc in range(2):
                            nc.tensor.matmul(
                                slots[p][:, oc * TQ : (oc + 1) * TQ],
                                lhsT=yt[ic][:, it * 128 : (it + 1) * 128],
                                rhs=wp[ic][:, oc * TQ : (oc + 1) * TQ],
                                start=False,
                                stop=(ic == 3),
                            )
                    ot = sp.tile([128, 2 * TQ], f32, name="otw", tag="otw",
                                 bufs=2)
                    for oc in range(2):
                        nc.vector.tensor_copy(
                            out=ot[:, oc * TQ : (oc + 1) * TQ],
                            in_=slots[p][:, oc * TQ : (oc + 1) * TQ],
                        )
                        nc.sync.dma_start(
                            out=out_d[it * 128 : (it + 1) * 128,
                                      oc * TQ : (oc + 1) * TQ],
                            in_=ot[:, oc * TQ : (oc + 1) * TQ],
                        )

                norm_tail = list(norm_tail)
                front_half(0)
                if norm_tail:
                    norm_tail.pop(0)()          # bcast+mul heads 4,5
                front_half(1)
                while norm_tail:
                    norm_tail.pop(0)()          # bcast+mul heads 6,7
                for p in range(2):
                    back_half(p)
                for p in range(2, 4):
                    front_half(p)
                    back_half(p)

            def post_pv(h, qc):
                """After the last PV of (h, qc): copy the PSUM denominator
                row into this head's row of the group's batched den tile,
                then copy the head's y into its half of the PAIR's packed
                [128,TQ] ysb tile (so normalization runs one full-width
                bcast+multiply per head pair)."""
                dens, slots = sweep_state[qc]
                yp = yps.pop(h)
                r0 = 32 * (h % 4)
                nc.vector.tensor_copy(
                    out=dens[h // 4][r0 : r0 + 1, :], in_=yp[64:65, :]
                )
                if h % 2 == 0:
                    pair_cur[h // 4] = smp.tile([128, TQ], f32, name="ysb2",
                                                tag="ysb", bufs=6)
                po = 64 * (h % 2)
                nc.vector.tensor_copy(
                    out=pair_cur[h // 4][po : po + 64, :], in_=yp[0:64, :]
                )
                if h % 2 == 1:
                    slots[h // 4].append(
                        ((h % 4) // 2, pair_cur.pop(h // 4))
                    )
                final = h == HL - 1 and qc == NQC - 1
                if h % 4 == 3 and not final:
                    fillers.extend(("norm", s) for s in norm_steps(h // 4, qc))
                if h == HL - 1:
                    if final:
                        # cover the last head's reciprocal latency with the
                        # reserved c_proj chains so the PE stays hot, then
                        # weave group 1's normalization into the first tail
                        # c_proj chains (whose pair-0/1 contractions only
                        # need the long-finished group-0 heads)
                        steps = norm_steps(1, qc)
                        steps[0](), steps[1]()      # recip + cast
                        for _, s in tail_reserve:
                            s()
                        tail_reserve.clear()
                        drain(8)
                        c_proj_tail(qc, steps[2:])
                    else:
                        for it in range(4 * qc, 4 * qc + 4):
                            for oc in range(2):
                                steps = c_proj_chain(it, oc)
                                if qc == NQC - 2 and it >= 4 * qc + 2:
                                    tail_reserve.extend(steps)
                                else:
                                    fillers.extend(steps)
                    del sweep_state[qc]

            def pv_group(h, qc, g):
                ktop = (qc + 1) * (TQ // TK)
                pt_w = pts.pop((h, qc, g))
                for hh in range(2):
                    ensure(f"v{h // 4}k{2 * g + hh}")
                for hh in range(2):
                    ktl = 2 * g + hh
                    j = ktl - qc * (TQ // TK)
                    col0 = j * TK if j >= 0 else 0
                    if ktl == 0:
                        yps[h] = psY.tile([D + 1, TQ], f32, name="yps",
                                          tag="y")
                    nc.tensor.matmul(
                        yps[h][:, col0:],
                        lhsT=vsb[ktl][:, h * 65 : (h + 1) * 65],
                        rhs=pt_w[:, hh * TQ + col0 : (hh + 1) * TQ],
                        start=(ktl == 0),
                        stop=(ktl == ktop - 1),
                    )
                if 2 * g + 1 == ktop - 1:
                    post_pv(h, qc)

            def s_group(h, qc, g):
                g2, po = h // 2, 64 * (h % 2)
                diag = 2 * g >= 4 * qc
                ps_s = psA.tile([128, 2 * TQ], f32, name="ps_s", tag="s_w")
                pt_w = ptp.tile([128, 2 * TQ], bf16, name="pt")
                for hh in range(2):
                    ktl = 2 * g + hh
                    j = ktl - qc * (TQ // TK)
                    col0 = j * TK if j >= 0 else 0
                    nc.tensor.matmul(
                        ps_s[:, hh * TQ + col0 : (hh + 1) * TQ],
                        lhsT=kt[g2][po : po + 64, ktl * TK : (ktl + 1) * TK],
                        rhs=qt[g2][po : po + 64,
                                   qc * TQ + col0 : (qc + 1) * TQ],
                        start=True,
                        stop=True,
                    )
                if not diag:
                    # one wide exp over both k-tiles (2 PSUM banks)
                    nc.scalar.activation(pt_w[:, :], ps_s[:, :], EXP,
                                         scale=0.125)
                else:
                    for hh in range(2):
                        ktl = 2 * g + hh
                        j = ktl - qc * (TQ // TK)
                        col0 = j * TK
                        nc.scalar.activation(
                            pt_w[:, hh * TQ + col0 : (hh + 1) * TQ],
                            ps_s[:, hh * TQ + col0 : (hh + 1) * TQ],
                            EXP,
                            scale=0.125,
                        )
                        nc.gpsimd.tensor_mul(
                            pt_w[:, hh * TQ + col0 : (hh + 1) * TQ],
                            pt_w[:, hh * TQ + col0 : (hh + 1) * TQ],
                            maskt[j][:, col0:],
                        )
                pts[(h, qc, g)] = pt_w
                return diag

            def tick():
                """After each S group emission: drip fillers, emit lagged PV
                groups (deeper lag for diagonal groups hides mask latency)."""
                drain(4 if len(fillers) > 160 else 3 if len(fillers) > 40
                      else 2)
                if pend and len(pend) >= (3 if pend[0][3] else 2):
                    h0, qc0, g0, _ = pend.pop(0)
                    pv_group(h0, qc0, g0)

            for qc in range(NQC):
                # flush previous sweeps' normalization before re-using the
                # rec/ysb slots (keeps the in-order DVE queue acyclic)
                ensure("norm")
                dens = [smp.tile([97, TQ], f32, name=f"den{i}", tag="deng",
                                 bufs=4) for i in range(2)]
                for dg in dens:
                    nc.vector.memset(dg[:], 1.0)  # rows between heads unused
                sweep_state[qc] = (dens, [[], []])
                for h in range(HL):
                    g2 = h // 2
                    ensure(f"qk{g2}j{qc}")
                    # spread V-tile prefetch over the heads whose PVs lag
                    if h in (0, 1, 4, 5):
                        half, o = h // 4, 2 * (h % 4)
                        ensure(f"v{half}k{4 * qc + o}")
                        ensure(f"v{half}k{4 * qc + o + 1}")
                    # prefetch next sweep's projection chunks while this
                    # sweep's exp stream can still hide the PE work
                    if qc < NQC - 1 and h >= 4:
                        ensure(f"qk{h - 4}j{qc + 1}")
                    for g in range((qc + 1) * (TQ // TK) // 2):
                        diag = s_group(h, qc, g)
                        pend.append((h, qc, g, diag))
                        tick()
            while pend:
                h0, qc0, g0, _ = pend.pop(0)
                pv_group(h0, qc0, g0)
                drain(1)

            drain(len(fillers))

    nc.finalize()
    return nc


def _bf16(a):
    return np.ascontiguousarray(a, dtype=np.float32).astype(ml_dtypes.bfloat16)


def _core_inputs(x, w_attn, b_attn, w_proj, masks, core):
    b, g = divmod(core, 2)
    gs = slice(g * CL, (g + 1) * CL)
    wq, wk, wv_ = (w_attn[i * C : (i + 1) * C][gs] for i in range(3))
    bq, bk = (b_attn[i * C : (i + 1) * C][gs] for i in range(2))

    # pair-interleaved QKV weight: col block 2p = Q pair p, 2p+1 = K pair p
    wqkT = np.empty((C, 2 * CL), np.float32)
    for p in range(4):
        wqkT[:, 256 * p : 256 * p + 128] = wq.T[:, 128 * p : 128 * (p + 1)]
        wqkT[:, 256 * p + 128 : 256 * (p + 1)] = wk.T[:, 128 * p : 128 * (p + 1)]
    # f32 bias columns, col 2p = Q pair p, col 2p+1 = K pair p
    bqk_cols = np.empty((128, 8), np.float32)
    for p in range(4):
        bqk_cols[:, 2 * p] = bq[128 * p : 128 * (p + 1)]
        bqk_cols[:, 2 * p + 1] = bk[128 * p : 128 * (p + 1)]

    return {
        "xt": _bf16(x[b].T).reshape(8, 128, T),
        "wqk": _bf16(wqkT).reshape(8, 128, 2 * CL),
        "wv": _bf16(wv_.T).reshape(8, 128, CL),
        "bqk": bqk_cols,
        "wp": _bf16(w_proj[:, gs].T).reshape(4, 128, C),
        "mask": masks,
    }


def _make_masks():
    qq = np.arange(TQ)[None, :]
    kk = np.arange(TK)[:, None]
    m = np.stack([(qq >= kk + j * TK) for j in range(4)]).astype(np.float32)
    return m.astype(ml_dtypes.bfloat16)


def kernel(x, w_attn, b_attn, w_proj, b_proj):
    global _prog, last_results
    from concourse.bass_utils import run_bass_kernel_spmd

    if _prog is None:
        _prog = _build_program()

    x = np.asarray(x, np.float32)
    w_attn = np.asarray(w_attn, np.float32)
    b_attn = np.asarray(b_attn, np.float32)
    w_proj = np.asarray(w_proj, np.float32)
    b_proj = np.asarray(b_proj, np.float32)

    masks = _make_masks()
    in_maps = [
        _core_inputs(x, w_attn, b_attn, w_proj, masks, core) for core in range(8)
    ]
    kwargs = {}
    tmpdir = os.environ.get("BASS_TMPDIR")
    if tmpdir:
        os.makedirs(tmpdir, exist_ok=True)
        kwargs["tmpdir"] = tmpdir
    res = run_bass_kernel_spmd(_prog, in_maps, list(range(8)), **kwargs)
    last_results = res

    # v-bias passes through attention as a constant (softmax rows sum to 1),
    # so its c_proj image is folded into the host-side bias add
    b_eff = b_proj + b_attn[2 * C :] @ w_proj.T
    out = np.empty((B, T, C), np.float32)
    for b in range(B):
        out[b] = res.results[2 * b]["out"] + res.results[2 * b + 1]["out"] + b_eff
    return out



# revision 47
# speedup vs baseline: 1.4488x; 1.0572x over previous
"""Causal self-attention (B=4, T=2048, C=1024, H=16) on 8 NeuronCores.

Sharding: core = (batch b, head-group g): data-parallel over B=4, tensor-
parallel over heads (2 groups x 8 heads).  Each core computes QKV + attention
for its 8 heads and the matching half of the c_proj contraction; the host
sums the two partial c_proj outputs per batch and adds b_proj.

Device layout notes:
  - all matmul operands bf16 (PE runs fp32 at 1/4 rate), PSUM f32
  - x, weights are pre-transposed on the host so every matmul contraction
    sits on the partition dim; no on-device transposes anywhere
  - QKV biases enter as K=1 rank-1 matmuls against a ones row
  - S is computed transposed ([keys, queries]); exp(S/8) on ScalarE with no
    max-subtraction (logits bounded ~+-4 for this problem's scale)
  - causality at tile granularity: k-tiles above the diagonal are skipped,
    diagonal tiles multiplied by precomputed 0/1 masks after exp; diagonal
    tiles further restrict S/exp/PV to their valid column range
  - softmax denominator = ones column appended to each head's V; PV matmul
    emits [y.T | denom] per (head, q-chunk)

Scheduling notes (the performance-critical part):
  - ScalarE exp is the pacer (~0.83ns/col + ~190ns/instr).  S tiles for two
    consecutive k-tiles are written into one [128,1024] 2-bank PSUM tile and
    exp'd with ONE activation for non-diagonal pairs (amortizes the fixed
    overhead); diagonal pairs keep two narrow column-sliced activations.
  - one continuous S->exp->PV pipeline per head across ALL q-chunks (PSUM
    yps double-buffered) so there is no PV-drain bubble at chunk boundaries;
    PV emission lags S by 2 groups (3 for diagonal groups, hiding the GPSIMD
    mask-multiply latency).
  - input DMAs are issued biases -> (wqk[k], xt[k]) pairs -> wv -> masks ->
    wp, and the pair-0 QKV projection is emitted k-innermost so the PE
    consumes tiles as they land; attention starts right after the pair-0
    Q/K projection + first 4 V tiles instead of after the full pre-phase.
  - remaining QKV projection work (pair 1-3 Q/K, V tiles) is drip-fed into
    the attention stream as PE "filler" (keeps the PE dense so the HAM
    activity monitor does not clock-throttle it to 1.2 GHz), with
    deadline-ordered tags force-flushed (ensure) just before a consumer.
  - normalization is per (head, q-chunk): reciprocal of the denominator row
    straight off the copied ysb tile, broadcast to 64 partitions via a
    rank-1 f32r matmul, multiply on DVE; the PE-side broadcast matmul is
    delayed one group so it never waits on the DVE reciprocal.
  - c_proj is emitted per q-chunk of the LAST head as filler work (all
    other pairs' y are normalized long before), so the output projection
    and its DMA overlap the tail of attention instead of serializing.
"""

import os

import numpy as np
import ml_dtypes

B, T, C, H = 4, 2048, 1024, 16
D = 64          # head dim
HL = 8          # heads per core
CL = HL * D     # 512 local channels
TQ = 512        # query chunk (matmul moving dim)
TK = 128        # key tile (psum partition dim)
NQC = T // TQ   # 4 query chunks
NKT = T // TK   # 16 key tiles
VW = HL * (D + 1)  # 520: V with per-head ones column

_prog = None
last_results = None  # BassKernelResults of the most recent run (for test.py)


def _build_program():
    import concourse.mybir as mybir
    import concourse.tile as tile
    from concourse import bacc

    f32 = mybir.dt.float32
    f32r = mybir.dt.float32r
    bf16 = mybir.dt.bfloat16
    EXP = mybir.ActivationFunctionType.Exp

    nc = bacc.Bacc("TRN2", target_bir_lowering=False, debug=False)

    xt_d = nc.dram_tensor("xt", [8, 128, T], bf16, kind="ExternalInput")
    # pair-interleaved: cols 256g..256g+128 = Q pair g, +128..256 = K pair g
    wqk_d = nc.dram_tensor("wqk", [8, 128, 2 * CL], bf16, kind="ExternalInput")
    wv_d = nc.dram_tensor("wv", [8, 128, CL], bf16, kind="ExternalInput")
    # per-pair bias columns, f32: col 2*g = Q pair g, col 2*g+1 = K pair g
    bqk_d = nc.dram_tensor("bqk", [128, 8], f32, kind="ExternalInput")
    wp_d = nc.dram_tensor("wp", [4, 128, C], bf16, kind="ExternalInput")
    mask_d = nc.dram_tensor("mask", [4, 128, TQ], bf16, kind="ExternalInput")
    out_d = nc.dram_tensor("out", [T, C], f32, kind="ExternalOutput")

    with tile.TileContext(nc) as tc:
        with (
            tc.tile_pool(name="persist", bufs=1) as pp,
            tc.tile_pool(name="ptpool", bufs=6) as ptp,
            tc.tile_pool(name="stage", bufs=4) as sp,
            tc.tile_pool(name="small", bufs=3) as smp,
            tc.tile_pool(name="psA", bufs=2, space="PSUM") as psA,   # [128,1024] x2 = 4 banks
            tc.tile_pool(name="psF", bufs=2, space="PSUM") as psF,   # [128,512]  x2 = 2 banks
            tc.tile_pool(name="psY", bufs=2, space="PSUM") as psY,   # [128,512]  x2 = 2 banks
        ):
            # ---- persistent SBUF tensors ----
            xt = [pp.tile([128, T], bf16, name=f"xt{k}") for k in range(8)]
            wqk = [pp.tile([128, 2 * CL], bf16, name=f"wqk{k}") for k in range(8)]
            wv = [pp.tile([128, CL], bf16, name=f"wv{k}") for k in range(8)]
            wp = [pp.tile([128, C], bf16, name=f"wp{k}") for k in range(4)]
            maskt = [pp.tile([128, TQ], bf16, name=f"mask{j}") for j in range(4)]
            bqk_cols = pp.tile([128, 8], f32, name="bqk_cols")
            # two-hot selector matrices: one matmul broadcasts rec rows 64*i
            # and 64*i+32 of a [97,512] tile into partitions 0-63 / 64-127 of
            # a [128,512] bc tile, normalizing a whole head PAIR at once
            # (SBUF APs may only start at partition 0/32/64/96, hence the
            # 32-pitch of the rec rows)
            sel2 = [pp.tile([97, 128], bf16, name=f"sel2_{i}") for i in range(2)]

            # DMA issue order = deadline order; the tile framework makes each
            # consumer wait only for its own slice, so compute overlaps the
            # load.  The pair-0/j-0 slices the pre-phase consumes are issued
            # as small leading chunks, split across the TWO hwdge issue
            # engines (SP + ACT run their ~600ns-per-dma issue streams in
            # parallel), so the first matmul starts ~5us earlier than a
            # whole-tile wait.
            for k in range(8):
                nc.scalar.dma_start(out=xt[k][:, 0:TQ], in_=xt_d[k][:, 0:TQ])
            nc.scalar.dma_start(out=bqk_cols[:], in_=bqk_d[:])
            for k in range(4):
                nc.scalar.dma_start(out=wv[k][:], in_=wv_d[k])
            for k in range(8):
                nc.sync.dma_start(out=wqk[k][:, 0:256], in_=wqk_d[k][:, 0:256])
            for k in range(4, 8):
                nc.sync.dma_start(out=wv[k][:], in_=wv_d[k])
            for j in range(4):
                nc.sync.dma_start(out=maskt[j][:], in_=mask_d[j])
            for k in range(8):
                nc.sync.dma_start(
                    out=wqk[k][:, 256:], in_=wqk_d[k][:, 256:]
                )
            for k in range(8):
                nc.sync.dma_start(out=xt[k][:, TQ : 2 * TQ],
                                  in_=xt_d[k][:, TQ : 2 * TQ])
            for k in range(4):
                nc.sync.dma_start(out=wp[k][:], in_=wp_d[k])
            for k in range(8):
                nc.sync.dma_start(out=xt[k][:, 2 * TQ :],
                                  in_=xt_d[k][:, 2 * TQ :])
            for i in range(2):
                nc.vector.memset(sel2[i][:], 0.0)
                nc.vector.memset(sel2[i][64 * i : 64 * i + 1, 0:64], 1.0)
                nc.vector.memset(sel2[i][64 * i + 32 : 64 * i + 33, 64:128],
                                 1.0)
            # warm-up: ~3.5us of dummy matmuls on a zeroed tile while the
            # first input DMAs land, so the HAM activity monitor has ramped
            # the PE to full clock (3us of continuous busy) before the real
            # instruction stream starts -- and stays there (no idle gap)
            warm = pp.tile([97, TQ], bf16, name="warm")
            nc.vector.memset(warm[:], 0.0)
            for _ in range(8):
                wps = psF.tile([128, TQ], f32, name="ps_warm", tag="fill")
                nc.tensor.matmul(wps[:], lhsT=warm[:, 0:128], rhs=warm[:],
                                 start=True, stop=True)

            # QT/KT in [channel, t] layout; channel tile g = head pair g
            qt = [pp.tile([128, T], bf16, name=f"qt{i}") for i in range(4)]
            kt = [pp.tile([128, T], bf16, name=f"kt{i}") for i in range(4)]
            # V in natural [t, channel] layout with a ones column per head
            vsb = [pp.tile([128, VW], bf16, name=f"v{i}") for i in range(NKT)]
            yt = [pp.tile([128, T], bf16, name=f"yt{i}") for i in range(4)]
            # softmax-denominator ones columns: written once here, V copies
            # below use a strided AP that skips them
            for it in range(NKT):
                v3 = vsb[it].rearrange("p (h c) -> p h c", h=HL)
                nc.vector.memset(v3[:, :, D : D + 1], 1.0)

            # ---- pair-0 Q/K j0 projection, k-innermost across the two halves
            # of one wide psA tile, so the PE consumes (wqk[k], xt[k]) as
            # they land from HBM
            def qk_wave(g, j0):
                slot = psA.tile([128, 2 * TQ], f32, name="ps_qk", tag="s_w")
                chs = [(qt, 0, j0), (kt, 1, j0)]
                for k in range(8):
                    for ci, (dst, qk, j) in enumerate(chs):
                        ps = slot[:, ci * TQ : (ci + 1) * TQ]
                        w0 = 256 * g + 128 * qk
                        nc.tensor.matmul(
                            ps,
                            lhsT=wqk[k][:, w0 : w0 + 128],
                            rhs=xt[k][:, j * TQ : (j + 1) * TQ],
                            start=(k == 0),
                            stop=(k == 7),
                        )
                for ci, (dst, qk, j) in enumerate(chs):
                    ps = slot[:, ci * TQ : (ci + 1) * TQ]
                    nc.vector.tensor_scalar_add(
                        dst[g][:, j * TQ : (j + 1) * TQ],
                        ps,
                        bqk_cols[:, 2 * g + qk : 2 * g + qk + 1],
                    )

            def emit_qk_filler(g):
                """One head-pair's Q.T and K.T projection as single-matmul
                closures, tagged per (pair, j) for deadline ensure()."""
                steps = []
                for j in range(NQC):
                    for dst, qk in ((qt, 0), (kt, 1)):
                        ph = {}

                        def step(k, ph=ph, dst=dst, qk=qk, j=j, g=g):
                            if k == 0:
                                ph["ps"] = psF.tile(
                                    [128, TQ], f32, name="ps_f", tag="fill"
                                )
                            if k < 8:
                                w0 = 256 * g + 128 * qk
                                nc.tensor.matmul(
                                    ph["ps"][:],
                                    lhsT=wqk[k][:, w0 : w0 + 128],
                                    rhs=xt[k][:, j * TQ : (j + 1) * TQ],
                                    start=(k == 0),
                                    stop=(k == 7),
                                )
                            else:
                                nc.vector.tensor_scalar_add(
                                    dst[g][:, j * TQ : (j + 1) * TQ],
                                    ph["ps"],
                                    bqk_cols[:, 2 * g + qk : 2 * g + qk + 1],
                                )

                        for k in range(9):
                            steps.append((f"qk{g}j{j}", lambda k=k, step=step: step(k)))
                return steps

            def v_chain_steps(it, h2):
                """V projection for 4 heads of one 128-row t-tile.  The copy
                out uses a 65-pitch strided AP that skips the preset ones
                columns (v bias is folded into b_proj on the host)."""
                ph = {}

                def step(k, ph=ph, it=it, h2=h2):
                    if k == 0:
                        ph["ps"] = psF.tile([128, TQ], f32, name="ps_v", tag="fill")
                    if k < 8:
                        nc.tensor.matmul(
                            ph["ps"][:, : CL // 2],
                            lhsT=xt[k][:, it * 128 : (it + 1) * 128],
                            rhs=wv[k][:, h2 * (CL // 2) : (h2 + 1) * (CL // 2)],
                            start=(k == 0),
                            stop=(k == 7),
                        )
                    else:
                        v3 = vsb[it].rearrange("p (h c) -> p h c", h=HL)
                        ps3 = ph["ps"].rearrange("p (h c) -> p h c", h=HL)
                        nc.vector.tensor_copy(
                            out=v3[:, 4 * h2 : 4 * h2 + 4, 0:D],
                            in_=ps3[:, 0:4, 0:D],
                        )

                return [(f"v{h2}k{it}", lambda k=k, step=step: step(k))
                        for k in range(9)]

            # pre-phase (inline, DMA-paced): pair-0 Q/K chunk j0 + the first
            # 4 V tiles of half 0 — the minimum sweep 0's first PVs consume
            qk_wave(0, 0)
            for it in range(4):
                for _, s in v_chain_steps(it, 0):
                    s()

            # ---- filler queue, deadline-ordered for the qc-major sweeps:
            # sweep s consumes chunk j=s of every pair and V t-tiles 4s..4s+3
            fillers = []
            qk_steps = {g: emit_qk_filler(g) for g in range(4)}
            for g in (1, 2):
                fillers.extend(qk_steps[g][0:18])         # qk{1,2}j0, due h2/h4
            for it in range(4):                           # V half1 0-3, due h4
                fillers.extend(v_chain_steps(it, 1))
            fillers.extend(qk_steps[3][0:18])             # qk3j0, due h6
            for s in range(1, NQC):
                for g in range(4):
                    fillers.extend(qk_steps[g][s * 18 : (s + 1) * 18])
                for it in range(4 * s, 4 * s + 4):
                    for h2 in range(2):
                        fillers.extend(v_chain_steps(it, h2))

            def drain(n):
                for _ in range(n):
                    if fillers:
                        fillers.pop(0)[1]()

            def ensure(tag):
                """Force-emit exactly the closures of `tag` (chains of
                different tags are mutually independent, so skipping others
                is safe and avoids serializing the whole backlog)."""
                keep, run = [], []
                for t, s in fillers:
                    (run if t == tag else keep).append((t, s))
                fillers[:] = keep
                for _, s in run:
                    s()

            # ---- attention: one continuous S->exp->PV stream per head ----
            def c_proj_chain(it, oc):
                """c_proj for one [128,512] output tile: 4-matmul chain +
                copy + DMA, as single-step closures."""
                ph = {}

                def step(s, ph=ph, it=it, oc=oc):
                    if s < 4:
                        if s == 0:
                            ph["ps"] = psF.tile([128, TQ], f32, name="ps_o",
                                                tag="fill")
                        nc.tensor.matmul(
                            ph["ps"][:],
                            lhsT=yt[s][:, it * 128 : (it + 1) * 128],
                            rhs=wp[s][:, oc * TQ : (oc + 1) * TQ],
                            start=(s == 0),
                            stop=(s == 3),
                        )
                    elif s == 4:
                        ph["ot"] = sp.tile([128, TQ], f32, name="ot")
                        nc.vector.tensor_copy(out=ph["ot"][:], in_=ph["ps"][:])
                    else:
                        nc.sync.dma_start(
                            out=out_d[it * 128 : (it + 1) * 128,
                                      oc * TQ : (oc + 1) * TQ],
                            in_=ph["ot"][:],
                        )

                return [("cp", lambda s=s, step=step: step(s)) for s in range(6)]

            # qc-major sweeps: process q-chunk qc for ALL 8 heads, then qc+1.
            # c_proj for a t-range starts one sweep after its columns are
            # final, so the output projection + DMA overlap attention instead
            # of serializing after it.
            pend = []     # (h, qc, g, diag) whose PV is not yet emitted
            yps = {}      # h -> current PSUM y accumulator [65, TQ]
            pts = {}      # (h, qc, g) -> pt tile
            sweep_state = {}  # qc -> (dens[2], slots[2])
            pair_cur = {}  # grp -> pair ysb tile being filled
            tail_reserve = []  # c_proj chains held for the final norm window

            def norm_steps(grp, qc):
                """Normalization of 4 heads: one full-tile approx reciprocal
                of the batched den tile (rows filled per head in post_pv,
                straight off the PSUM denominator row), one f32->bf16 cast,
                then per-head broadcast+multiply closures.  bf16 rec keeps
                the rank-1 broadcast matmul at full bf16 rate."""
                dens, slots = sweep_state[qc]
                den_g, group = dens[grp], slots[grp]
                rec_g = smp.tile([97, TQ], f32, name="rec_g", tag="recg",
                                 bufs=3)
                rec_b = smp.tile([97, TQ], bf16, name="rec_b", tag="recb",
                                 bufs=3)
                steps = [
                    lambda: nc.vector.reciprocal_approx_fast(
                        out=rec_g[:], in_=den_g[:]),
                    lambda: nc.vector.tensor_copy(out=rec_b[:], in_=rec_g[:]),
                ]

                def one(p, ysb2, qc=qc, rec_b=rec_b, grp=grp):
                    bc = psF.tile([128, TQ], f32, name="bc", tag="fill")
                    nc.tensor.matmul(
                        bc[:], lhsT=sel2[p][:], rhs=rec_b[:],
                        start=True, stop=True,
                    )
                    nc.vector.tensor_mul(
                        yt[2 * grp + p][:, qc * TQ : (qc + 1) * TQ],
                        ysb2[:],
                        bc[:],
                    )

                for p, ysb2 in group:
                    steps.append(lambda p=p, ysb2=ysb2: one(p, ysb2))
                return steps

            def c_proj_tail(qc, norm_tail):
                """Final sweep's c_proj on wide PSUM (the S pipeline is done
                with psA by now).  Pairs 0/1 of the contraction only need
                group-0 heads (normalized long ago), so two tiles' worth of
                those matmuls are emitted first and the final group's
                broadcast+multiply steps (`norm_tail`) are woven between
                them to hide the normalization round-trip latency."""
                slots = {}

                def front_half(p):
                    it = 4 * qc + p
                    slots[p] = psA.tile([128, 2 * TQ], f32, name="ps_cp",
                                        tag="s_w")
                    for ic in range(2):
                        for oc in range(2):
                            nc.tensor.matmul(
                                slots[p][:, oc * TQ : (oc + 1) * TQ],
                                lhsT=yt[ic][:, it * 128 : (it + 1) * 128],
                                rhs=wp[ic][:, oc * TQ : (oc + 1) * TQ],
                                start=(ic == 0),
                                stop=False,
                            )

                def back_half(p):
                    it = 4 * qc + p
                    for ic in range(2, 4):
                        for oc in range(2):
                            nc.tensor.matmul(
                                slots[p][:, oc * TQ : (oc + 1) * TQ],
                                lhsT=yt[ic][:, it * 128 : (it + 1) * 128],
                                rhs=wp[ic][:, oc * TQ : (oc + 1) * TQ],
                                start=False,
                                stop=(ic == 3),
                            )
                    ot = sp.tile([128, 2 * TQ], f32, name="otw", tag="otw",
                                 bufs=2)
                    for oc in range(2):
                        nc.vector.tensor_copy(
                            out=ot[:, oc * TQ : (oc + 1) * TQ],
                            in_=slots[p][:, oc * TQ : (oc + 1) * TQ],
                        )
                        nc.sync.dma_start(
                            out=out_d[it * 128 : (it + 1) * 128,
                                      oc * TQ : (oc + 1) * TQ],
                            in_=ot[:, oc * TQ : (oc + 1) * TQ],
                        )

                norm_tail = list(norm_tail)
                front_half(0)
                if norm_tail:
                    norm_tail.pop(0)()          # bcast+mul heads 4,5
                front_half(1)
                while norm_tail:
                    norm_tail.pop(0)()          # bcast+mul heads 6,7
                for p in range(2):
                    back_half(p)
                for p in range(2, 4):
                    front_half(p)
                    back_half(p)

            def post_pv(h, qc):
                """After the last PV of (h, qc): copy the PSUM denominator
                row into this head's row of the group's batched den tile,
                then copy the head's y into its half of the PAIR's packed
                [128,TQ] ysb tile (so normalization runs one full-width
                bcast+multiply per head pair)."""
                dens, slots = sweep_state[qc]
                yp = yps.pop(h)
                r0 = 32 * (h % 4)
                nc.vector.tensor_copy(
                    out=dens[h // 4][r0 : r0 + 1, :], in_=yp[64:65, :]
                )
                if h % 2 == 0:
                    pair_cur[h // 4] = smp.tile([128, TQ], f32, name="ysb2",
                                                tag="ysb", bufs=6)
                po = 64 * (h % 2)
                nc.vector.tensor_copy(
                    out=pair_cur[h // 4][po : po + 64, :], in_=yp[0:64, :]
                )
                if h % 2 == 1:
                    slots[h // 4].append(
                        ((h % 4) // 2, pair_cur.pop(h // 4))
                    )
                final = h == HL - 1 and qc == NQC - 1
                if h % 4 == 3 and not final:
                    fillers.extend(("norm", s) for s in norm_steps(h // 4, qc))
                if h == HL - 1:
                    if final:
                        # cover the last head's reciprocal latency with the
                        # reserved c_proj chains so the PE stays hot, then
                        # weave group 1's normalization into the first tail
                        # c_proj chains (whose pair-0/1 contractions only
                        # need the long-finished group-0 heads)
                        steps = norm_steps(1, qc)
                        steps[0](), steps[1]()      # recip + cast
                        for _, s in tail_reserve:
                            s()
                        tail_reserve.clear()
                        drain(8)
                        c_proj_tail(qc, steps[2:])
                    else:
                        for it in range(4 * qc, 4 * qc + 4):
                            for oc in range(2):
                                steps = c_proj_chain(it, oc)
                                if qc == NQC - 2 and it >= 4 * qc + 2:
                                    tail_reserve.extend(steps)
                                else:
                                    fillers.extend(steps)
                    del sweep_state[qc]

            def pv_group(pair, qc, ktl):
                """PV for BOTH heads of a pair from one [128,1024] pt tile
                (head-even in cols 0:512, head-odd in 512:1024)."""
                ktop = (qc + 1) * (TQ // TK)
                pt_w = pts.pop((pair, qc, ktl))
                ensure(f"v{pair // 2}k{ktl}")
                j = ktl - qc * (TQ // TK)
                col0 = j * TK if j >= 0 else 0
                for hh in range(2):
                    h = 2 * pair + hh
                    if ktl == 0:
                        yps[h] = psY.tile([D + 1, TQ], f32, name="yps",
                                          tag="y")
                    nc.tensor.matmul(
                        yps[h][:, col0:],
                        lhsT=vsb[ktl][:, h * 65 : (h + 1) * 65],
                        rhs=pt_w[:, hh * TQ + col0 : (hh + 1) * TQ],
                        start=(ktl == 0),
                        stop=(ktl == ktop - 1),
                    )
                if ktl == ktop - 1:
                    post_pv(2 * pair, qc)
                    post_pv(2 * pair + 1, qc)

            def s_group(pair, qc, ktl):
                """S for BOTH heads of a pair for ONE k-tile, as two K=64
                matmuls row-tiled onto disjoint halves of the PE array
                (heads of a pair live on partitions 0-63 / 64-127 of the
                same qt/kt tiles, so the two matmuls run concurrently --
                one 512-column stream instead of two)."""
                j = ktl - qc * (TQ // TK)
                col0 = j * TK if j >= 0 else 0
                diag = ktl >= 4 * qc
                ps_s = psA.tile([128, 2 * TQ], f32, name="ps_s", tag="s_w")
                pt_w = ptp.tile([128, 2 * TQ], bf16, name="pt")
                for hh in range(2):
                    po = 64 * hh
                    nc.tensor.matmul(
                        ps_s[:, hh * TQ + col0 : (hh + 1) * TQ],
                        lhsT=kt[pair][po : po + 64,
                                      ktl * TK : (ktl + 1) * TK],
                        rhs=qt[pair][po : po + 64,
                                     qc * TQ + col0 : (qc + 1) * TQ],
                        start=True,
                        stop=True,
                        tile_position=(po, 0),
                    )
                if not diag:
                    # one wide exp over both heads (2 PSUM banks)
                    nc.scalar.activation(pt_w[:, :], ps_s[:, :], EXP,
                                         scale=0.125)
                else:
                    for hh in range(2):
                        nc.scalar.activation(
                            pt_w[:, hh * TQ + col0 : (hh + 1) * TQ],
                            ps_s[:, hh * TQ + col0 : (hh + 1) * TQ],
                            EXP,
                            scale=0.125,
                        )
                        nc.gpsimd.tensor_mul(
                            pt_w[:, hh * TQ + col0 : (hh + 1) * TQ],
                            pt_w[:, hh * TQ + col0 : (hh + 1) * TQ],
                            maskt[j][:, col0:],
                        )
                pts[(pair, qc, ktl)] = pt_w
                return diag

            def tick():
                """After each S group emission: drip fillers, emit lagged PV
                groups (deeper lag for diagonal groups hides mask latency)."""
                drain(4 if len(fillers) > 160 else 3 if len(fillers) > 40
                      else 2)
                if pend and len(pend) >= (4 if pend[0][3] else 3):
                    p0, qc0, k0, _ = pend.pop(0)
                    pv_group(p0, qc0, k0)

            for qc in range(NQC):
                # flush previous sweeps' normalization before re-using the
                # rec/ysb slots (keeps the in-order DVE queue acyclic)
                ensure("norm")
                dens = [smp.tile([97, TQ], f32, name=f"den{i}", tag="deng",
                                 bufs=4) for i in range(2)]
                for dg in dens:
                    nc.vector.memset(dg[:], 1.0)  # rows between heads unused
                sweep_state[qc] = (dens, [[], []])
                for pair in range(4):
                    ensure(f"qk{pair}j{qc}")
                    # spread V-tile prefetch over the pairs whose PVs lag
                    half, o = pair // 2, 2 * (pair % 2)
                    ensure(f"v{half}k{4 * qc + o}")
                    ensure(f"v{half}k{4 * qc + o + 1}")
                    # prefetch next sweep's projection chunks while this
                    # sweep's exp stream can still hide the PE work
                    if qc < NQC - 1 and pair >= 2:
                        for gg in (2 * (pair - 2), 2 * (pair - 2) + 1):
                            ensure(f"qk{gg}j{qc + 1}")
                    for ktl in range((qc + 1) * (TQ // TK)):
                        diag = s_group(pair, qc, ktl)
                        pend.append((pair, qc, ktl, diag))
                        tick()
            while pend:
                p0, qc0, k0, _ = pend.pop(0)
                pv_group(p0, qc0, k0)
                drain(1)

            drain(len(fillers))

    nc.finalize()
    return nc


def _bf16(a):
    return np.ascontiguousarray(a, dtype=np.float32).astype(ml_dtypes.bfloat16)


def _core_inputs(x, w_attn, b_attn, w_proj, masks, core):
    b, g = divmod(core, 2)
    gs = slice(g * CL, (g + 1) * CL)
    wq, wk, wv_ = (w_attn[i * C : (i + 1) * C][gs] for i in range(3))
    bq, bk = (b_attn[i * C : (i + 1) * C][gs] for i in range(2))

    # pair-interleaved QKV weight: col block 2p = Q pair p, 2p+1 = K pair p
    wqkT = np.empty((C, 2 * CL), np.float32)
    for p in range(4):
        wqkT[:, 256 * p : 256 * p + 128] = wq.T[:, 128 * p : 128 * (p + 1)]
        wqkT[:, 256 * p + 128 : 256 * (p + 1)] = wk.T[:, 128 * p : 128 * (p + 1)]
    # f32 bias columns, col 2p = Q pair p, col 2p+1 = K pair p
    bqk_cols = np.empty((128, 8), np.float32)
    for p in range(4):
        bqk_cols[:, 2 * p] = bq[128 * p : 128 * (p + 1)]
        bqk_cols[:, 2 * p + 1] = bk[128 * p : 128 * (p + 1)]

    return {
        "xt": _bf16(x[b].T).reshape(8, 128, T),
        "wqk": _bf16(wqkT).reshape(8, 128, 2 * CL),
        "wv": _bf16(wv_.T).reshape(8, 128, CL),
        "bqk": bqk_cols,
        "wp": _bf16(w_proj[:, gs].T).reshape(4, 128, C),
        "mask": masks,
    }


def _make_masks():
    qq = np.arange(TQ)[None, :]
    kk = np.arange(TK)[:, None]
    m = np.stack([(qq >= kk + j * TK) for j in range(4)]).astype(np.float32)
    return m.astype(ml_dtypes.bfloat16)


def kernel(x, w_attn, b_attn, w_proj, b_proj):
    global _prog, last_results
    from concourse.bass_utils import run_bass_kernel_spmd

    if _prog is None:
        _prog = _build_program()

    x = np.asarray(x, np.float32)
    w_attn = np.asarray(w_attn, np.float32)
    b_attn = np.asarray(b_attn, np.float32)
    w_proj = np.asarray(w_proj, np.float32)
    b_proj = np.asarray(b_proj, np.float32)

    masks = _make_masks()
    in_maps = [
        _core_inputs(x, w_attn, b_attn, w_proj, masks, core) for core in range(8)
    ]
    kwargs = {}
    tmpdir = os.environ.get("BASS_TMPDIR")
    if tmpdir:
        os.makedirs(tmpdir, exist_ok=True)
        kwargs["tmpdir"] = tmpdir
    res = run_bass_kernel_spmd(_prog, in_maps, list(range(8)), **kwargs)
    last_results = res

    # v-bias passes through attention as a constant (softmax rows sum to 1),
    # so its c_proj image is folded into the host-side bias add
    b_eff = b_proj + b_attn[2 * C :] @ w_proj.T
    out = np.empty((B, T, C), np.float32)
    for b in range(B):
        out[b] = res.results[2 * b]["out"] + res.results[2 * b + 1]["out"] + b_eff
    return out

